# revision 2
# baseline (speedup 1.0000x reference)
"""Signature-kernel Gram matrix on 8 NeuronCores — v2.

Math per pair (x (128,8), y (128,8)):
  K = exp(x@y.T - 0.5|x|^2 - 0.5|y|^2)           RBF gram, sigma=1
  diff = second mixed finite difference of K      (127,127)
  Goursat PDE on the dyadic-refined fine grid G (255,255), G[0,:]=G[:,0]=1,
    G[i,j] = c1*(G[i-1,j]+G[i,j-1]) - c2*G[i-1,j-1]
    with inc = diff/4 constant on 2x2 fine blocks,
    c1 = 1 + diff/8 + diff^2/192, c2 = 1 - diff^2/192
  answer = G[254,254]

v2 structure (one core = 32 pairs = 2 local xs × 16 ys):
  A: load/transpose sequences, build stacked-K matmul operands
     LH[10,256] = [x^T; -|x|^2/2; 1], RH[10,2048] = [Y^T; 1; -|y|^2/2]
  B: per x-row a: one matmul (K=10) -> exponent, ACT exp,
     col-diff (DVE), row-shift (PE shf matmul), DIFF (DVE)
  C: c1-1 (fp16) and r-1 = c2/c1-1 (fp16) in [127, 16x127] layout
  D: SBUF->SBUF DMA gathers into pair-per-partition, 4-way column-block
     split, pre-skewed by 2 rows per block group
  E: wavefront PDE: 260 steps; step T runs block-group g at fine row
     T-2g on partitions 32g+p. Per step (DVE): t=r*k0, m=k1-t,
     scan state=(m+state)*c1 (one scan over all 128 partitions);
     boundary hand-off via two [96,1] copies (mod-3 rotating buffers).
     Coefficient rows expanded fp16->fp32 on ACT every other step.

Sharding: data-parallel over batch_x: core c owns x rows {2c, 2c+1} x all
16 ys. Host gathers the (16,16) output.
"""

import numpy as np
from contextlib import ExitStack

import concourse.bass as bass
import concourse.bacc as bacc
import concourse.tile as tile
from concourse import mybir
from concourse.bass_utils import run_bass_kernel_spmd

F32 = mybir.dt.float32
F16 = mybir.dt.float16
AL = mybir.AluOpType
AF = mybir.ActivationFunctionType

N_CORES = 8
L = 128          # sequence length
D = 8            # feature dim
NY = 16          # ys per core
NX = 2           # xs per core
NP = NX * NY     # 32 pairs per core
M = L - 1        # 127 coarse grid
G = 2 * M        # 254 fine grid (dyadic order 1)
NSEQ = NX + NY   # 18
NSTEP = G + 6    # wavefront steps (4 groups, skew 2)
SLOTS = NSTEP // 2 + 1  # coarse slots incl skew pad


def _rep2(ap):
    """[P, n] view -> [P, n, 2] with zero-stride inner dim."""
    return bass.AP(tensor=ap.tensor, offset=ap.offset,
                   ap=[ap.ap[0], ap.ap[1], [0, 2]])


def _build(upto="full", nstep=NSTEP):
    nc = bacc.Bacc()
    xs_t = nc.dram_tensor("xs", [NX * L, D], F32, kind="ExternalInput")
    ys_t = nc.dram_tensor("ys", [NY * L, D], F32, kind="ExternalInput")
    idn_t = nc.dram_tensor("idn", [L, L], F32, kind="ExternalInput")
    shf_t = nc.dram_tensor("shf", [L, L], F32, kind="ExternalInput")
    shp_t = nc.dram_tensor("shp", [L, L], F32, kind="ExternalInput")
    out_t = nc.dram_tensor("out", [NP, 1], F32, kind="ExternalOutput")

    rings = None  # set after pools

    with ExitStack() as ctx:
        tc = ctx.enter_context(tile.TileContext(nc))
        constp = ctx.enter_context(tc.tile_pool(name="constp", bufs=1))
        iop = ctx.enter_context(tc.tile_pool(name="iop", bufs=3))
        workp = ctx.enter_context(tc.tile_pool(name="workp", bufs=2))
        chp = ctx.enter_context(tc.tile_pool(name="chp", bufs=1))
        ep = ctx.enter_context(tc.tile_pool(name="ep", bufs=2))

        rings = [nc.sync, nc.scalar, nc.gpsimd]

        # ---- Phase A ----
        idn_s = iop.tile([L, L], F32, tag="idn_s")
        nc.sync.dma_start(out=idn_s, in_=idn_t[:, :])
        idn = constp.tile([L, L], F32)
        nc.vector.tensor_copy(idn, idn_s)
        shp_s = iop.tile([L, L], F32, tag="shp_s")
        nc.gpsimd.dma_start(out=shp_s, in_=shp_t[:, :])
        shp = constp.tile([L, L], F32)
        nc.vector.tensor_copy(shp, shp_s)
        ones8 = constp.tile([D, 1], F32)
        nc.vector.memset(ones8, 1.0)
        one1 = constp.tile([1, 1], F32)
        nc.vector.memset(one1, 1.0)
        e0row = constp.tile([1, L], F32)
        nc.vector.memset(e0row, 0.0)
        nc.vector.memset(e0row[:, 0:32], 1.0)

        # LH rows: 0-7 x^T, 8 = -0.5|x|^2, 9 = ones   (cols: a*L..)
        # RH rows: 0-7 y^T, 8 = ones, 9 = -0.5|y|^2   (cols: b*L..)
        LH = constp.tile([D + 2, NX * L], F32)
        RH = constp.tile([D + 2, NY * L], F32)
        ones2k = constp.tile([1, NY * L], F32)
        nc.vector.memset(ones2k, 1.0)
        # rows 8/9 are written via DMA (compute ops must start at partition 0)
        nc.sync.dma_start(out=LH[D + 1 : D + 2, :], in_=ones2k[:, 0 : NX * L])
        nc.scalar.dma_start(out=RH[D : D + 1, :], in_=ones2k[:, :])

        with tc.tile_pool(name="psA", bufs=1, space="PSUM") as psA, \
             tc.tile_pool(name="awork", bufs=1) as awork:
            # batched sequence loads: [i-partition, (seq, feat)]
            xr_s = awork.tile([L, NX * D], F32, tag="xr_s")
            nc.scalar.dma_start(
                out=xr_s, in_=xs_t.rearrange("(a i) k -> i a k", a=NX)
            )
            xr = awork.tile([L, NX * D], F32, tag="xr")
            nc.vector.tensor_copy(xr, xr_s)
            yr_s = awork.tile([L, NY * D], F32, tag="yr_s")
            nc.sync.dma_start(
                out=yr_s, in_=ys_t.rearrange("(b i) k -> i b k", b=NY)
            )
            yr = awork.tile([L, NY * D], F32, tag="yr")
            nc.vector.tensor_copy(yr, yr_s)
            # transposes: 4 per PSUM bank tile, one ACT copy per bank
            psx = psA.tile([D, NX * L], F32, tag="psx", bufs=1)
            for a in range(NX):
                nc.tensor.transpose(
                    psx[:, a * L : (a + 1) * L], xr[:, a * D : (a + 1) * D], idn
                )
            nc.scalar.activation(LH[0:D, :], psx, AF.Copy)
            for yb in range(4):
                psy = psA.tile([D, 4 * L], F32, tag="psy", bufs=2)
                for j in range(4):
                    b = 4 * yb + j
                    nc.tensor.transpose(
                        psy[:, j * L : (j + 1) * L],
                        yr[:, b * D : (b + 1) * D], idn,
                    )
                nc.scalar.activation(
                    RH[0:D, yb * 512 : (yb + 1) * 512], psy, AF.Copy
                )

            # norms (computed in base-0 tiles, DMA'd into LH/RH rows 8/9)
            sqx = awork.tile([D, NX * L], F32, tag="sqx")
            nc.scalar.square(sqx, LH[0:D, :])
            nxp = psA.tile([1, NX * L], F32, tag="nxp", bufs=1)
            nc.tensor.matmul(nxp, ones8, sqx)
            nxrow = awork.tile([1, NX * L], F32, tag="nxrow")
            nc.scalar.activation(nxrow, nxp, AF.Copy, scale=-0.5)
            nc.gpsimd.dma_start(out=LH[D : D + 1, :], in_=nxrow[:, :])
            sqy = awork.tile([D, NY * L], F32, tag="sqy")
            nc.scalar.square(sqy, RH[0:D, :])
            nyrow = awork.tile([1, NY * L], F32, tag="nyrow")
            for blk in range(4):
                nyp = psA.tile([1, 512], F32, tag="nyp", bufs=2)
                nc.tensor.matmul(nyp, ones8, sqy[:, blk * 512 : (blk + 1) * 512])
                nc.scalar.activation(
                    nyrow[:, blk * 512 : (blk + 1) * 512],
                    nyp, AF.Copy, scale=-0.5,
                )
            nc.sync.dma_start(out=RH[D + 1 : D + 2, :], in_=nyrow[:, :])

        # coefficient staging tiles (per half a): [127p, 16 pairs * 127]
        W = NY * M  # 2032
        c1h = [None, None]
        rm1h = [None, None]

        if upto == "A":
            nc.sync.dma_start(out=out_t[0:2, :], in_=LH[0:2, 0:1])

        # ---- Phases B + C, two 8-pair chunks per half, pipelined ----
        # Exponent AND row-shifted exponent via two matmul sets (f32r);
        # diff = col-diff of (Ksh - K). Engine balance: Pool does the two
        # subs, ACT exps/squares/copies, DVE the coefficient math.
        F32R = mybir.dt.float32r
        psp_cm = tc.tile_pool(name="psp", bufs=1, space="PSUM")
        psp = psp_cm.__enter__()
        CW = NY * L // 2   # 1024 exp cols per chunk
        CWM = W // 2       # 1016 coef cols per chunk
        c1h = {}
        rm1h = {}
        for a in range(NX if upto != "A" else 0):
            for cb in range(2):
                kps = psp.tile([L, CW], F32, tag="kps", bufs=2)  # 2 banks
                kpsh = psp.tile([M, CW], F32, tag="kpsh", bufs=2)
                for blk in range(2):
                    rsl = slice(cb * CW + blk * 512, cb * CW + (blk + 1) * 512)
                    osl = slice(blk * 512, (blk + 1) * 512)
                    nc.tensor.matmul(
                        kps[:, osl],
                        LH[:, a * L : (a + 1) * L],
                        RH[:, rsl],
                    )
                    nc.tensor.matmul(
                        kpsh[:, osl],
                        LH[:, a * L + 1 : (a + 1) * L],
                        RH[:, rsl],
                    )
                ke = workp.tile([L, CW], F32, tag="ke", bufs=2)
                nc.scalar.activation(ke, kps, AF.Exp)
                kesh = workp.tile([M, CW], F32, tag="kesh", bufs=2)
                nc.scalar.activation(kesh, kpsh, AF.Exp)
                ed = workp.tile([M, CW], F32, tag="ed", bufs=2)
                nc.gpsimd.tensor_sub(ed, kesh, ke[0:M, :])
                edv = ed.rearrange("p (b j) -> p b j", b=NY // 2)
                diff = workp.tile([M, CWM], F32, tag="diff", bufs=2)
                nc.vector.tensor_sub(
                    diff.rearrange("p (b j) -> p b j", b=NY // 2),
                    edv[:, :, 1:L], edv[:, :, 0:M],
                )
                # qb = (diff/sqrt(192))^2 ; c1m1 = diff/8 + qb (fp16)
                # rec = 1/(1+c1m1) ; n = qb + c1m1 ; rm1 = -n*rec (fp16)
                qb = workp.tile([M, CWM], F32, tag="qb", bufs=2)
                nc.scalar.activation(
                    qb, diff, AF.Square, scale=1.0 / np.sqrt(192.0)
                )
                c1 = workp.tile([M, CWM], F16, tag="c1m1", bufs=4)
                nc.vector.scalar_tensor_tensor(
                    c1, diff, 0.125, qb, AL.mult, AL.add
                )
                c1f = workp.tile([M, CWM], F32, tag="c1f", bufs=2)
                nc.scalar.activation(c1f, c1, AF.Copy, bias=1.0)
                rec = workp.tile([M, CWM], F32, tag="rec", bufs=2)
                nc.vector.reciprocal(rec, c1f)
                n = workp.tile([M, CWM], F32, tag="n", bufs=2)
                nc.gpsimd.tensor_add(n, qb, c1)
                rm = workp.tile([M, CWM], F16, tag="rm1", bufs=4)
                nc.vector.scalar_tensor_tensor(
                    rm, n, -1.0, rec, AL.mult, AL.mult
                )
                c1h[(a, cb)] = c1
                rm1h[(a, cb)] = rm

        # ---- Phase D: DRAM bounce into skewed pair-major layout ----
        # CHC/CHR [128, SLOTS*32] fp16; partition 32g+16a+b; group g covers
        # coarse cols 32g..32g+31 (g=3: 31 + zero pad); slot k holds coarse
        # row k-g. Stores are contiguous row-major (127 descs of 4KB); loads
        # gather the per-group column slices.
        if upto == "C":
            nc.sync.dma_start(
                out=out_t[:, :], in_=c1h[(1, 1)][0:NP, 0:2].bitcast(F32)
            )
        if upto in ("A", "C"):
            nc.finalize_after_pools = True  # sentinel unused; early build
        else:
            dramp = ctx.enter_context(
                tc.tile_pool(name="dramp", bufs=1, space="DRAM")
            )
            # lo/hi split by slot so phase E can start while hi loads/expands
            SLO = 66  # slots 0..65 in lo tiles, 66..SLOTS-1 in hi
            CHt = {}
            for nm2, part, nsl in (("c", "lo", SLO), ("c", "hi", SLOTS - SLO),
                                   ("r", "lo", SLO), ("r", "hi", SLOTS - SLO)):
                t = chp.tile([4 * NP, nsl * 32], F16, tag=f"CH{nm2}_{part}")
                nc.vector.memset(t, 0.0)
                CHt[(nm2, part)] = t
            di = 0
            drt = {}
            for a in range(NX):
                for nm, arrs in (("c1", c1h), ("rm", rm1h)):
                    d = dramp.tile([M, W], F16, tag=f"d_{nm}{a}")
                    for cb in range(2):
                        rings[di % 3].dma_start(
                            out=d[:, cb * CWM : (cb + 1) * CWM],
                            in_=arrs[(a, cb)][0:M, :],
                        )
                        di += 1
                    drt[(nm, a)] = d
            for part in ("lo", "hi"):
                for g in range(4):
                    w = 32 if g < 3 else 31
                    # slot k holds coarse row k-g; lo covers slots g..SLO-1
                    if part == "lo":
                        r0, r1 = 0, SLO - g        # coarse rows
                        base = 0
                    else:
                        r0, r1 = SLO - g, M
                        base = SLO
                    for a in range(NX):
                        for nm2, nm in (("c", "c1"), ("r", "rm")):
                            CH = CHt[(nm2, part)]
                            d = drt[(nm, a)]
                            dst = CH[
                                32 * g + 16 * a : 32 * g + 16 * a + 16,
                                (g + r0 - base) * 32 : (g + r1 - base) * 32,
                            ].rearrange("p (r c) -> p r c", c=32)[:, :, 0:w]
                            src = d.rearrange("r (q c) -> q r c", q=NY)[
                                :, r0:r1, 32 * g : 32 * g + w
                            ]
                            rings[di % 3].dma_start(out=dst, in_=src)
                            di += 1

        # ---- Phase E: 4-group wavefront ----
        psp_cm.__exit__(None, None, None)
        if upto == "D":
            nc.sync.dma_start(
                out=out_t[:, :], in_=CHt[("c", "hi")][0:NP, 0:2].bitcast(F32)
            )
        if upto in ("A", "C", "D"):
            nstep = 0
        else:
            psE = ctx.enter_context(
                tc.tile_pool(name="psE", bufs=1, space="PSUM")
            )
            # bulk coefficient expansion: fp16 (c-1) -> fp32 c, repeat2
            # lo/hi expansion tiles; hi expands while phase E runs on lo
            pc1_lo = chp.tile([4 * NP, SLO * 64], F32, tag="PC1_lo")
            pc1_hi = chp.tile([4 * NP, (SLOTS - SLO) * 64], F32, tag="PC1_hi")
            pr_lo = chp.tile([4 * NP, SLO * 64], F32, tag="PR_lo")
            pr_hi = chp.tile([4 * NP, (SLOTS - SLO) * 64], F32, tag="PR_hi")
            PC1 = {"lo": pc1_lo, "hi": pc1_hi}
            PR = {"lo": pr_lo, "hi": pr_hi}
            for part, nsl in (("lo", SLO), ("hi", SLOTS - SLO)):
                ncch = 2
                csz = (nsl + ncch - 1) // ncch
                for ch in range(ncch):
                    s0 = ch * csz
                    s1 = min(nsl, s0 + csz)
                    nc.scalar.activation(
                        PC1[part][:, s0 * 64 : s1 * 64].rearrange(
                            "p (a b) -> p a b", b=2),
                        _rep2(CHt[("c", part)][:, s0 * 32 : s1 * 32]),
                        AF.Copy, bias=1.0,
                    )
                    nc.scalar.activation(
                        PR[part][:, s0 * 64 : s1 * 64].rearrange(
                            "p (a b) -> p a b", b=2),
                        _rep2(CHt[("r", part)][:, s0 * 32 : s1 * 32]),
                        AF.Copy, bias=1.0,
                    )
            GA = constp.tile([4 * NP, 66], F32)
            GB = constp.tile([4 * NP, 66], F32)
            nc.vector.memset(GA, 1.0)
            nc.vector.memset(GB, 1.0)
            B3 = []
            for k in range(3):
                b = psE.tile([4 * NP, 1], F32, tag=f"b3_{k}")
                # initialize boundary buffers to 1.0 via the ones matmul
                nc.tensor.matmul(b, e0row, one1, start=True, stop=False)
                nc.tensor.matmul(
                    b, shp, GA[:, 64:65], start=False, stop=True
                )
                B3.append(b)

        for T in range(1, nstep + 1):
            slot = (T - 1) // 2
            part = "lo" if slot < SLO else "hi"
            sl = slot if part == "lo" else slot - SLO
            exp_c = PC1[part][:, sl * 64 : (sl + 1) * 64]
            exp_r = PR[part][:, sl * 64 : (sl + 1) * 64]
            cur, prv = (GA, GB) if T % 2 == 1 else (GB, GA)
            kb = T % 3
            kread = (T + 1) % 3  # holds boundary produced at step T-2
            # boundary -> cur col 0 (k0 for next step's mul; scan initial)
            nc.vector.tensor_copy(cur[:, 0:1], B3[kread][:, 0:1])
            t64 = ep.tile([4 * NP, 64], F32, tag="t64")
            m = ep.tile([4 * NP, 64], F32, tag="m")
            nc.vector.tensor_mul(t64, exp_r, prv[:, 0:64])
            nc.vector.tensor_sub(m, prv[:, 1:65], t64)
            nc.vector.tensor_tensor_scan(
                cur[:, 1:65], m, exp_c, cur[:, 0:1], AL.add, AL.mult
            )
            # boundary out via PE shift: B3[kb] = shp^T. @ cur[:,64] + e0
            nc.tensor.matmul(B3[kb], e0row, one1, start=True, stop=False)
            nc.tensor.matmul(B3[kb], shp, cur[:, 64:65], start=False, stop=True)

        if nstep > 0:
            final = GB if nstep % 2 == 0 else GA
            nc.sync.dma_start(out=out_t[:, :], in_=final[96:128, 62:63])

    nc.finalize()
    return nc


_CACHE = {}


def _get_nc():
    if "nc" not in _CACHE:
        _CACHE["nc"] = _build()
    return _CACHE["nc"]


def run(xs, ys, trace=False):
    xs = np.ascontiguousarray(np.asarray(xs), dtype=np.float32)
    ys = np.ascontiguousarray(np.asarray(ys), dtype=np.float32)
    assert xs.shape == (16, L, D) and ys.shape == (16, L, D)
    nc = _get_nc()
    idn = np.eye(L, dtype=np.float32)
    shf = np.eye(L, k=-1, dtype=np.float32)
    shp = np.eye(L, k=32, dtype=np.float32)
    in_maps = []
    for c in range(N_CORES):
        in_maps.append(
            {
                "xs": xs[2 * c : 2 * c + 2].reshape(NX * L, D).copy(),
                "ys": ys.reshape(NY * L, D).copy(),
                "idn": idn,
                "shf": shf,
                "shp": shp,
            }
        )
    try:
        res = run_bass_kernel_spmd(nc, in_maps, list(range(N_CORES)), trace=trace)
    except ModuleNotFoundError:
        res = run_bass_kernel_spmd(nc, in_maps, list(range(N_CORES)), trace=False)
    rows = [res.results[c]["out"].reshape(NX, NY) for c in range(N_CORES)]
    out = np.concatenate(rows, axis=0)
    return out, res


def kernel(xs, ys):
    out, _ = run(xs, ys)
    return out


# revision 3
# speedup vs baseline: 1.0263x; 1.0263x over previous
"""Signature-kernel Gram matrix on 8 NeuronCores.

Math per pair (x (128,8), y (128,8)):
  K = exp(x@y.T - 0.5|x|^2 - 0.5|y|^2)           RBF gram, sigma=1
  diff = second mixed finite difference of K      (127,127)
  Goursat PDE on the dyadic-refined fine grid G (255,255), G[0,:]=G[:,0]=1,
    G[i,j] = c1*(G[i-1,j]+G[i,j-1]) - c2*G[i-1,j-1]
    with inc = diff/4 constant on 2x2 fine blocks,
    c1 = 1 + diff/8 + diff^2/192, c2 = 1 - diff^2/192
  answer = G[254,254]

Structure (one core = 32 pairs = 2 local xs x 16 ys):
  A: batched loads, PE transposes, stacked matmul operands
     LH[10,256] = [x^T; -|x|^2/2; 1], RH[10,2048] = [y^T; 1; -|y|^2/2]
     (norms via ACT square + DVE reduce + one PE transpose).
  B: per (x-row a, 8-pair chunk): one K=10 matmul for the exponent AND
     one for the row-shifted exponent (lhsT column-offset by 1), ACT exp;
     diff = col-diff of (Ksh - K) on DVE.
  C: c1-1 and r-1 = c2/c1-1 as fp16 (values are tiny, so fp16 on the
     DELTAS keeps ~1e-5 abs precision); reciprocal on DVE.
  D: flatten to pair-major via DRAM bounce: contiguous row-major stores
     (127 descriptors x 4KB), then strided per-column-block loads into
     pre-skewed CH tiles (lo/hi slot split so phase E starts early).
     Layout: partition 32g+16a+b owns pair (a,b), column block g; slot k
     of the CH array holds coarse row k-g (skew 2 rows per group).
  E: bulk-expand c1/r rows (repeat2, +1.0 bias, fp32) once on ACT, then
     254+6 wavefront steps. Step T runs block-group g at fine row T-2g:
     DVE: copy boundary->cur[:,0], t=r*k0, m=k1-t,
          scan state=(m+state)*c1 over all 128 partitions (scan initial
          = cur[:,0]); PE shifts scan-end columns +32 partitions into a
          rotating PSUM buffer (plus an e0 matmul for group 0's 1.0).
  Output: group 3 partitions, local column 62 = G[254,254] per pair.

Sharding: data-parallel over batch_x: core c owns x rows {2c, 2c+1} x all
16 ys. Host gathers the (16,16) output.
"""

import numpy as np
from contextlib import ExitStack

import concourse.bass as bass
import concourse.bacc as bacc
import concourse.tile as tile
from concourse import mybir
from concourse.bass_utils import run_bass_kernel_spmd

F32 = mybir.dt.float32
F16 = mybir.dt.float16
AL = mybir.AluOpType
AF = mybir.ActivationFunctionType

N_CORES = 8
L = 128          # sequence length
D = 8            # feature dim
NY = 16          # ys per core
NX = 2           # xs per core
NP = NX * NY     # 32 pairs per core
M = L - 1        # 127 coarse grid
G = 2 * M        # 254 fine grid (dyadic order 1)
NSEQ = NX + NY   # 18
NSTEP = G + 6    # wavefront steps (4 groups, skew 2)
SLOTS = NSTEP // 2 + 1  # coarse slots incl skew pad


def _rep2(ap):
    """[P, n] view -> [P, n, 2] with zero-stride inner dim."""
    return bass.AP(tensor=ap.tensor, offset=ap.offset,
                   ap=[ap.ap[0], ap.ap[1], [0, 2]])


def _build(upto="full", nstep=NSTEP):
    nc = bacc.Bacc()
    xs_t = nc.dram_tensor("xs", [NX * L, D], F32, kind="ExternalInput")
    ys_t = nc.dram_tensor("ys", [NY * L, D], F32, kind="ExternalInput")
    idn_t = nc.dram_tensor("idn", [L, L], F32, kind="ExternalInput")
    shf_t = nc.dram_tensor("shf", [L, L], F32, kind="ExternalInput")
    shp_t = nc.dram_tensor("shp", [L, L], F32, kind="ExternalInput")
    out_t = nc.dram_tensor("out", [NP, 1], F32, kind="ExternalOutput")

    rings = None  # set after pools

    with ExitStack() as ctx:
        tc = ctx.enter_context(tile.TileContext(nc))
        constp = ctx.enter_context(tc.tile_pool(name="constp", bufs=1))
        iop = ctx.enter_context(tc.tile_pool(name="iop", bufs=3))
        workp = ctx.enter_context(tc.tile_pool(name="workp", bufs=2))
        chp = ctx.enter_context(tc.tile_pool(name="chp", bufs=1))
        ep = ctx.enter_context(tc.tile_pool(name="ep", bufs=2))

        rings = [nc.sync, nc.scalar, nc.gpsimd]

        # ---- Phase A ----
        idn_s = iop.tile([L, L], F32, tag="idn_s")
        nc.sync.dma_start(out=idn_s, in_=idn_t[:, :])
        idn = constp.tile([L, L], F32)
        nc.vector.tensor_copy(idn, idn_s)
        shp_s = iop.tile([L, L], F32, tag="shp_s")
        nc.gpsimd.dma_start(out=shp_s, in_=shp_t[:, :])
        shp = constp.tile([L, L], F32)
        nc.vector.tensor_copy(shp, shp_s)
        ones8 = constp.tile([D, 1], F32)
        nc.vector.memset(ones8, 1.0)
        one1 = constp.tile([1, 1], F32)
        nc.vector.memset(one1, 1.0)
        e0row = constp.tile([1, L], F32)
        nc.vector.memset(e0row, 0.0)
        nc.vector.memset(e0row[:, 0:32], 1.0)

        # LH rows: 0-7 x^T, 8 = -0.5|x|^2, 9 = ones   (cols: a*L..)
        # RH rows: 0-7 y^T, 8 = ones, 9 = -0.5|y|^2   (cols: b*L..)
        LH = constp.tile([D + 2, NX * L], F32)
        RH = constp.tile([D + 2, NY * L], F32)
        ones2k = constp.tile([1, NY * L], F32)
        nc.vector.memset(ones2k, 1.0)
        # rows 8/9 are written via DMA (compute ops must start at partition 0)
        nc.sync.dma_start(out=LH[D + 1 : D + 2, :], in_=ones2k[:, 0 : NX * L])
        nc.scalar.dma_start(out=RH[D : D + 1, :], in_=ones2k[:, :])

        with tc.tile_pool(name="psA", bufs=1, space="PSUM") as psA, \
             tc.tile_pool(name="awork", bufs=1) as awork:
            # batched sequence loads: [i-partition, (seq, feat)]
            xr_s = awork.tile([L, NX * D], F32, tag="xr_s")
            nc.scalar.dma_start(
                out=xr_s, in_=xs_t.rearrange("(a i) k -> i a k", a=NX)
            )
            xr = awork.tile([L, NX * D], F32, tag="xr")
            nc.vector.tensor_copy(xr, xr_s)
            yr_s = awork.tile([L, NY * D], F32, tag="yr_s")
            nc.sync.dma_start(
                out=yr_s, in_=ys_t.rearrange("(b i) k -> i b k", b=NY)
            )
            yr = awork.tile([L, NY * D], F32, tag="yr")
            nc.vector.tensor_copy(yr, yr_s)
            # transposes: 4 per PSUM bank tile, one ACT copy per bank
            psx = psA.tile([D, NX * L], F32, tag="psx", bufs=1)
            for a in range(NX):
                nc.tensor.transpose(
                    psx[:, a * L : (a + 1) * L], xr[:, a * D : (a + 1) * D], idn
                )
            nc.scalar.activation(LH[0:D, :], psx, AF.Copy)
            for yb in range(4):
                psy = psA.tile([D, 4 * L], F32, tag="psy", bufs=2)
                for j in range(4):
                    b = 4 * yb + j
                    nc.tensor.transpose(
                        psy[:, j * L : (j + 1) * L],
                        yr[:, b * D : (b + 1) * D], idn,
                    )
                nc.scalar.activation(
                    RH[0:D, yb * 512 : (yb + 1) * 512], psy, AF.Copy
                )

            # norms: square+reduce in sequence-index layout, one transpose,
            # then DMA rows into LH/RH
            sq = awork.tile([L, (NX + NY) * D], F32, tag="sq")
            nc.scalar.square(sq[:, 0 : NY * D], yr)
            nc.scalar.square(sq[:, NY * D :], xr)
            nr = awork.tile([L, NX + NY], F32, tag="nr")
            nc.vector.tensor_reduce(
                nr, sq.rearrange("p (s k) -> p s k", k=D),
                mybir.AxisListType.X, AL.add,
            )
            nrp = psA.tile([NX + NY, L], F32, tag="nrp", bufs=1)
            nc.tensor.transpose(nrp, nr, idn)
            nrt = awork.tile([NX + NY, L], F32, tag="nrt")
            nc.scalar.activation(nrt, nrp, AF.Copy, scale=-0.5)
            nc.sync.dma_start(out=RH[D + 1 : D + 2, :], in_=nrt[0:NY, :])
            nc.gpsimd.dma_start(out=LH[D : D + 1, :], in_=nrt[NY : NY + NX, :])

        # coefficient staging tiles (per half a): [127p, 16 pairs * 127]
        W = NY * M  # 2032
        c1h = [None, None]
        rm1h = [None, None]

        if upto == "A":
            nc.sync.dma_start(out=out_t[0:2, :], in_=LH[0:2, 0:1])

        # ---- Phases B + C, two 8-pair chunks per half, pipelined ----
        # Exponent AND row-shifted exponent via two matmul sets (f32r);
        # diff = col-diff of (Ksh - K). Engine balance: Pool does the two
        # subs, ACT exps/squares/copies, DVE the coefficient math.
        F32R = mybir.dt.float32r
        psp_cm = tc.tile_pool(name="psp", bufs=1, space="PSUM")
        psp = psp_cm.__enter__()
        CW = NY * L // 2   # 1024 exp cols per chunk
        CWM = W // 2       # 1016 coef cols per chunk
        c1h = {}
        rm1h = {}
        for a in range(NX if upto != "A" else 0):
            for cb in range(2):
                kps = psp.tile([L, CW], F32, tag="kps", bufs=2)  # 2 banks
                kpsh = psp.tile([M, CW], F32, tag="kpsh", bufs=2)
                for blk in range(2):
                    rsl = slice(cb * CW + blk * 512, cb * CW + (blk + 1) * 512)
                    osl = slice(blk * 512, (blk + 1) * 512)
                    nc.tensor.matmul(
                        kps[:, osl],
                        LH[:, a * L : (a + 1) * L],
                        RH[:, rsl],
                    )
                    nc.tensor.matmul(
                        kpsh[:, osl],
                        LH[:, a * L + 1 : (a + 1) * L],
                        RH[:, rsl],
                    )
                ke = workp.tile([L, CW], F32, tag="ke", bufs=2)
                nc.scalar.activation(ke, kps, AF.Exp)
                kesh = workp.tile([M, CW], F32, tag="kesh", bufs=2)
                nc.scalar.activation(kesh, kpsh, AF.Exp)
                ed = workp.tile([M, CW], F32, tag="ed", bufs=2)
                nc.vector.tensor_sub(ed, kesh, ke[0:M, :])
                edv = ed.rearrange("p (b j) -> p b j", b=NY // 2)
                diff = workp.tile([M, CWM], F32, tag="diff", bufs=2)
                nc.vector.tensor_sub(
                    diff.rearrange("p (b j) -> p b j", b=NY // 2),
                    edv[:, :, 1:L], edv[:, :, 0:M],
                )
                # qb = (diff/sqrt(192))^2 ; c1m1 = diff/8 + qb (fp16)
                # rec = 1/(1+c1m1) ; n = qb + c1m1 ; rm1 = -n*rec (fp16)
                qb = workp.tile([M, CWM], F32, tag="qb", bufs=2)
                nc.scalar.activation(
                    qb, diff, AF.Square, scale=1.0 / np.sqrt(192.0)
                )
                c1 = workp.tile([M, CWM], F16, tag="c1m1", bufs=4)
                nc.vector.scalar_tensor_tensor(
                    c1, diff, 0.125, qb, AL.mult, AL.add
                )
                c1f = workp.tile([M, CWM], F32, tag="c1f", bufs=2)
                nc.scalar.activation(c1f, c1, AF.Copy, bias=1.0)
                rec = workp.tile([M, CWM], F32, tag="rec", bufs=2)
                nc.vector.reciprocal(rec, c1f)
                n = workp.tile([M, CWM], F32, tag="n", bufs=2)
                nc.gpsimd.tensor_add(n, qb, c1)
                rm = workp.tile([M, CWM], F16, tag="rm1", bufs=4)
                nc.vector.scalar_tensor_tensor(
                    rm, n, -1.0, rec, AL.mult, AL.mult
                )
                c1h[(a, cb)] = c1
                rm1h[(a, cb)] = rm

        # ---- Phase D: DRAM bounce into skewed pair-major layout ----
        # CHC/CHR [128, SLOTS*32] fp16; partition 32g+16a+b; group g covers
        # coarse cols 32g..32g+31 (g=3: 31 + zero pad); slot k holds coarse
        # row k-g. Stores are contiguous row-major (127 descs of 4KB); loads
        # gather the per-group column slices.
        if upto == "C":
            nc.sync.dma_start(
                out=out_t[:, :], in_=c1h[(1, 1)][0:NP, 0:2].bitcast(F32)
            )
        if upto in ("A", "C"):
            nc.finalize_after_pools = True  # sentinel unused; early build
        else:
            dramp = ctx.enter_context(
                tc.tile_pool(name="dramp", bufs=1, space="DRAM")
            )
            # lo/hi split by slot so phase E can start while hi loads/expands
            SLO = 66  # slots 0..65 in lo tiles, 66..SLOTS-1 in hi
            CHt = {}
            for nm2, part, nsl in (("c", "lo", SLO), ("c", "hi", SLOTS - SLO),
                                   ("r", "lo", SLO), ("r", "hi", SLOTS - SLO)):
                t = chp.tile([4 * NP, nsl * 32], F16, tag=f"CH{nm2}_{part}")
                nc.vector.memset(t, 0.0)
                CHt[(nm2, part)] = t
            di = 0
            drt = {}
            for a in range(NX):
                for nm, arrs in (("c1", c1h), ("rm", rm1h)):
                    d = dramp.tile([M, W], F16, tag=f"d_{nm}{a}")
                    for cb in range(2):
                        rings[di % 3].dma_start(
                            out=d[:, cb * CWM : (cb + 1) * CWM],
                            in_=arrs[(a, cb)][0:M, :],
                        )
                        di += 1
                    drt[(nm, a)] = d
            for part in ("lo", "hi"):
                for g in range(4):
                    w = 32 if g < 3 else 31
                    # slot k holds coarse row k-g; lo covers slots g..SLO-1
                    if part == "lo":
                        r0, r1 = 0, SLO - g        # coarse rows
                        base = 0
                    else:
                        r0, r1 = SLO - g, M
                        base = SLO
                    for a in range(NX):
                        for nm2, nm in (("c", "c1"), ("r", "rm")):
                            CH = CHt[(nm2, part)]
                            d = drt[(nm, a)]
                            dst = CH[
                                32 * g + 16 * a : 32 * g + 16 * a + 16,
                                (g + r0 - base) * 32 : (g + r1 - base) * 32,
                            ].rearrange("p (r c) -> p r c", c=32)[:, :, 0:w]
                            src = d.rearrange("r (q c) -> q r c", q=NY)[
                                :, r0:r1, 32 * g : 32 * g + w
                            ]
                            rings[di % 3].dma_start(out=dst, in_=src)
                            di += 1

        # ---- Phase E: 4-group wavefront ----
        psp_cm.__exit__(None, None, None)
        if upto == "D":
            nc.sync.dma_start(
                out=out_t[:, :], in_=CHt[("c", "hi")][0:NP, 0:2].bitcast(F32)
            )
        if upto in ("A", "C", "D"):
            nstep = 0
        else:
            psE = ctx.enter_context(
                tc.tile_pool(name="psE", bufs=1, space="PSUM")
            )
            # bulk coefficient expansion: fp16 (c-1) -> fp32 c, repeat2
            # lo/hi expansion tiles; hi expands while phase E runs on lo
            pc1_lo = chp.tile([4 * NP, SLO * 64], F32, tag="PC1_lo")
            pc1_hi = chp.tile([4 * NP, (SLOTS - SLO) * 64], F32, tag="PC1_hi")
            pr_lo = chp.tile([4 * NP, SLO * 64], F32, tag="PR_lo")
            pr_hi = chp.tile([4 * NP, (SLOTS - SLO) * 64], F32, tag="PR_hi")
            PC1 = {"lo": pc1_lo, "hi": pc1_hi}
            PR = {"lo": pr_lo, "hi": pr_hi}
            for part, nsl in (("lo", SLO), ("hi", SLOTS - SLO)):
                ncch = 2
                csz = (nsl + ncch - 1) // ncch
                for ch in range(ncch):
                    s0 = ch * csz
                    s1 = min(nsl, s0 + csz)
                    nc.scalar.activation(
                        PC1[part][:, s0 * 64 : s1 * 64].rearrange(
                            "p (a b) -> p a b", b=2),
                        _rep2(CHt[("c", part)][:, s0 * 32 : s1 * 32]),
                        AF.Copy, bias=1.0,
                    )
                    nc.scalar.activation(
                        PR[part][:, s0 * 64 : s1 * 64].rearrange(
                            "p (a b) -> p a b", b=2),
                        _rep2(CHt[("r", part)][:, s0 * 32 : s1 * 32]),
                        AF.Copy, bias=1.0,
                    )
            GA = constp.tile([4 * NP, 66], F32)
            GB = constp.tile([4 * NP, 66], F32)
            nc.vector.memset(GA, 1.0)
            nc.vector.memset(GB, 1.0)
            B3 = []
            for k in range(3):
                b = psE.tile([4 * NP, 1], F32, tag=f"b3_{k}")
                # initialize boundary buffers to 1.0 via the ones matmul
                nc.tensor.matmul(b, e0row, one1, start=True, stop=False)
                nc.tensor.matmul(
                    b, shp, GA[:, 64:65], start=False, stop=True
                )
                B3.append(b)

        for T in range(1, nstep + 1):
            slot = (T - 1) // 2
            part = "lo" if slot < SLO else "hi"
            sl = slot if part == "lo" else slot - SLO
            exp_c = PC1[part][:, sl * 64 : (sl + 1) * 64]
            exp_r = PR[part][:, sl * 64 : (sl + 1) * 64]
            cur, prv = (GA, GB) if T % 2 == 1 else (GB, GA)
            kb = T % 3
            kread = (T + 1) % 3  # holds boundary produced at step T-2
            # boundary -> cur col 0 (k0 for next step's mul; scan initial)
            nc.vector.tensor_copy(cur[:, 0:1], B3[kread][:, 0:1])
            t64 = ep.tile([4 * NP, 64], F32, tag="t64")
            m = ep.tile([4 * NP, 64], F32, tag="m")
            nc.vector.tensor_mul(t64, exp_r, prv[:, 0:64])
            nc.vector.tensor_sub(m, prv[:, 1:65], t64)
            nc.vector.tensor_tensor_scan(
                cur[:, 1:65], m, exp_c, cur[:, 0:1], AL.add, AL.mult
            )
            # boundary out via PE shift: B3[kb] = shp^T. @ cur[:,64] + e0
            nc.tensor.matmul(B3[kb], e0row, one1, start=True, stop=False)
            nc.tensor.matmul(B3[kb], shp, cur[:, 64:65], start=False, stop=True)

        if nstep > 0:
            final = GB if nstep % 2 == 0 else GA
            nc.sync.dma_start(out=out_t[:, :], in_=final[96:128, 62:63])

    nc.finalize()
    return nc


_CACHE = {}


def _get_nc():
    if "nc" not in _CACHE:
        _CACHE["nc"] = _build()
    return _CACHE["nc"]


def run(xs, ys, trace=False):
    xs = np.ascontiguousarray(np.asarray(xs), dtype=np.float32)
    ys = np.ascontiguousarray(np.asarray(ys), dtype=np.float32)
    assert xs.shape == (16, L, D) and ys.shape == (16, L, D)
    nc = _get_nc()
    idn = np.eye(L, dtype=np.float32)
    shf = np.eye(L, k=-1, dtype=np.float32)
    shp = np.eye(L, k=32, dtype=np.float32)
    in_maps = []
    for c in range(N_CORES):
        in_maps.append(
            {
                "xs": xs[2 * c : 2 * c + 2].reshape(NX * L, D).copy(),
                "ys": ys.reshape(NY * L, D).copy(),
                "idn": idn,
                "shf": shf,
                "shp": shp,
            }
        )
    try:
        res = run_bass_kernel_spmd(nc, in_maps, list(range(N_CORES)), trace=trace)
    except ModuleNotFoundError:
        res = run_bass_kernel_spmd(nc, in_maps, list(range(N_CORES)), trace=False)
    rows = [res.results[c]["out"].reshape(NX, NY) for c in range(N_CORES)]
    out = np.concatenate(rows, axis=0)
    return out, res


def kernel(xs, ys):
    out, _ = run(xs, ys)
    return out


# revision 4
# speedup vs baseline: 1.0388x; 1.0122x over previous
"""Signature-kernel Gram matrix on 8 NeuronCores.

Math per pair (x (128,8), y (128,8)):
  K = exp(x@y.T - 0.5|x|^2 - 0.5|y|^2)           RBF gram, sigma=1
  diff = second mixed finite difference of K      (127,127)
  Goursat PDE on the dyadic-refined fine grid G (255,255), G[0,:]=G[:,0]=1,
    G[i,j] = c1*(G[i-1,j]+G[i,j-1]) - c2*G[i-1,j-1]
    with inc = diff/4 constant on 2x2 fine blocks,
    c1 = 1 + diff/8 + diff^2/192, c2 = 1 - diff^2/192
  answer = G[254,254]

Structure (one core = 32 pairs = 2 local xs x 16 ys):
  A: batched loads, PE transposes, stacked matmul operands
     LH[10,256] = [x^T; -|x|^2/2; 1], RH[10,2048] = [y^T; 1; -|y|^2/2]
     (norms via ACT square + DVE reduce + one PE transpose).
  B: per (x-row a, 8-pair chunk): one K=10 matmul for the exponent AND
     one for the row-shifted exponent (lhsT column-offset by 1), ACT exp;
     diff = col-diff of (Ksh - K) on DVE.
  C: c1-1 and r-1 = c2/c1-1 as fp16 (values are tiny, so fp16 on the
     DELTAS keeps ~1e-5 abs precision); reciprocal on DVE.
  D: flatten to pair-major via DRAM bounce: contiguous row-major stores
     (127 descriptors x 4KB), then strided per-column-block loads into
     pre-skewed CH tiles (lo/hi slot split so phase E starts early).
     Layout: partition 32g+16a+b owns pair (a,b), column block g; slot k
     of the CH array holds coarse row k-g (skew 2 rows per group).
  E: bulk-expand c1/r rows (repeat2, +1.0 bias, fp32) once on ACT, then
     254+6 wavefront steps. Step T runs block-group g at fine row T-2g:
     DVE: copy boundary->cur[:,0], t=r*k0, m=k1-t,
          scan state=(m+state)*c1 over all 128 partitions (scan initial
          = cur[:,0]); PE shifts scan-end columns +32 partitions into a
          rotating PSUM buffer (plus an e0 matmul for group 0's 1.0).
  Output: group 3 partitions, local column 62 = G[254,254] per pair.

Sharding: data-parallel over batch_x: core c owns x rows {2c, 2c+1} x all
16 ys. Host gathers the (16,16) output.
"""

import numpy as np
from contextlib import ExitStack

import concourse.bass as bass
import concourse.bacc as bacc
import concourse.tile as tile
from concourse import mybir
from concourse.bass_utils import run_bass_kernel_spmd

F32 = mybir.dt.float32
F16 = mybir.dt.float16
AL = mybir.AluOpType
AF = mybir.ActivationFunctionType

N_CORES = 8
L = 128          # sequence length
D = 8            # feature dim
NY = 16          # ys per core
NX = 2           # xs per core
NP = NX * NY     # 32 pairs per core
M = L - 1        # 127 coarse grid
G = 2 * M        # 254 fine grid (dyadic order 1)
NSEQ = NX + NY   # 18
NSTEP = G + 6    # wavefront steps (4 groups, skew 2)
SLOTS = NSTEP // 2 + 1  # coarse slots incl skew pad


def _rep2(ap):
    """[P, n] view -> [P, n, 2] with zero-stride inner dim."""
    return bass.AP(tensor=ap.tensor, offset=ap.offset,
                   ap=[ap.ap[0], ap.ap[1], [0, 2]])


def _build(upto="full", nstep=NSTEP):
    nc = bacc.Bacc()
    xs_t = nc.dram_tensor("xs", [NX * L, D], F32, kind="ExternalInput")
    ys_t = nc.dram_tensor("ys", [NY * L, D], F32, kind="ExternalInput")
    idn_t = nc.dram_tensor("idn", [L, L], F32, kind="ExternalInput")
    shf_t = nc.dram_tensor("shf", [L, L], F32, kind="ExternalInput")
    shp_t = nc.dram_tensor("shp", [L, L], F32, kind="ExternalInput")
    out_t = nc.dram_tensor("out", [NP, 1], F32, kind="ExternalOutput")

    rings = None  # set after pools

    with ExitStack() as ctx:
        tc = ctx.enter_context(tile.TileContext(nc))
        constp = ctx.enter_context(tc.tile_pool(name="constp", bufs=1))
        iop = ctx.enter_context(tc.tile_pool(name="iop", bufs=3))
        workp = ctx.enter_context(tc.tile_pool(name="workp", bufs=2))
        chp = ctx.enter_context(tc.tile_pool(name="chp", bufs=1))
        ep = ctx.enter_context(tc.tile_pool(name="ep", bufs=2))

        rings = [nc.sync, nc.scalar, nc.gpsimd]

        # ---- Phase A ----
        idn_s = iop.tile([L, L], F32, tag="idn_s")
        nc.sync.dma_start(out=idn_s, in_=idn_t[:, :])
        idn = constp.tile([L, L], F32)
        nc.vector.tensor_copy(idn, idn_s)
        shp_s = iop.tile([L, L], F32, tag="shp_s")
        nc.gpsimd.dma_start(out=shp_s, in_=shp_t[:, :])
        shp = constp.tile([L, L], F32)
        nc.vector.tensor_copy(shp, shp_s)
        ones8 = constp.tile([D, 1], F32)
        nc.vector.memset(ones8, 1.0)
        one1 = constp.tile([1, 1], F32)
        nc.vector.memset(one1, 1.0)
        e0row = constp.tile([1, L], F32)
        nc.vector.memset(e0row, 0.0)
        nc.vector.memset(e0row[:, 0:32], 1.0)

        # LH rows: 0-7 x^T, 8 = -0.5|x|^2, 9 = ones   (cols: a*L..)
        # RH rows: 0-7 y^T, 8 = ones, 9 = -0.5|y|^2   (cols: b*L..)
        LH = constp.tile([D + 2, NX * L], F32)
        RH = constp.tile([D + 2, NY * L], F32)
        ones2k = constp.tile([1, NY * L], F32)
        nc.vector.memset(ones2k, 1.0)
        # rows 8/9 are written via DMA (compute ops must start at partition 0)
        nc.sync.dma_start(out=LH[D + 1 : D + 2, :], in_=ones2k[:, 0 : NX * L])
        nc.scalar.dma_start(out=RH[D : D + 1, :], in_=ones2k[:, :])

        with tc.tile_pool(name="psA", bufs=1, space="PSUM") as psA, \
             tc.tile_pool(name="awork", bufs=1) as awork:
            # batched sequence loads: [i-partition, (seq, feat)]
            xr_s = awork.tile([L, NX * D], F32, tag="xr_s")
            nc.scalar.dma_start(
                out=xr_s, in_=xs_t.rearrange("(a i) k -> i a k", a=NX)
            )
            xr = awork.tile([L, NX * D], F32, tag="xr")
            nc.vector.tensor_copy(xr, xr_s)
            yr_s = awork.tile([L, NY * D], F32, tag="yr_s")
            nc.sync.dma_start(
                out=yr_s, in_=ys_t.rearrange("(b i) k -> i b k", b=NY)
            )
            yr = awork.tile([L, NY * D], F32, tag="yr")
            nc.vector.tensor_copy(yr, yr_s)
            # transposes: 4 per PSUM bank tile, one ACT copy per bank
            psx = psA.tile([D, NX * L], F32, tag="psx", bufs=1)
            for a in range(NX):
                nc.tensor.transpose(
                    psx[:, a * L : (a + 1) * L], xr[:, a * D : (a + 1) * D], idn
                )
            nc.scalar.activation(LH[0:D, :], psx, AF.Copy)
            for yb in range(4):
                psy = psA.tile([D, 4 * L], F32, tag="psy", bufs=2)
                for j in range(4):
                    b = 4 * yb + j
                    nc.tensor.transpose(
                        psy[:, j * L : (j + 1) * L],
                        yr[:, b * D : (b + 1) * D], idn,
                    )
                nc.scalar.activation(
                    RH[0:D, yb * 512 : (yb + 1) * 512], psy, AF.Copy
                )

            # norms: square+reduce in sequence-index layout, one transpose,
            # then DMA rows into LH/RH
            sq = awork.tile([L, (NX + NY) * D], F32, tag="sq")
            nc.scalar.square(sq[:, 0 : NY * D], yr)
            nc.scalar.square(sq[:, NY * D :], xr)
            nr = awork.tile([L, NX + NY], F32, tag="nr")
            nc.vector.tensor_reduce(
                nr, sq.rearrange("p (s k) -> p s k", k=D),
                mybir.AxisListType.X, AL.add,
            )
            nrp = psA.tile([NX + NY, L], F32, tag="nrp", bufs=1)
            nc.tensor.transpose(nrp, nr, idn)
            nrt = awork.tile([NX + NY, L], F32, tag="nrt")
            nc.scalar.activation(nrt, nrp, AF.Copy, scale=-0.5)
            nc.sync.dma_start(out=RH[D + 1 : D + 2, :], in_=nrt[0:NY, :])
            nc.gpsimd.dma_start(out=LH[D : D + 1, :], in_=nrt[NY : NY + NX, :])

        # coefficient staging tiles (per half a): [127p, 16 pairs * 127]
        W = NY * M  # 2032
        c1h = [None, None]
        rm1h = [None, None]

        if upto == "A":
            nc.sync.dma_start(out=out_t[0:2, :], in_=LH[0:2, 0:1])

        # ---- Phases B + C, two 8-pair chunks per half, pipelined ----
        # Exponent AND row-shifted exponent via two matmul sets (f32r);
        # diff = col-diff of (Ksh - K). Engine balance: Pool does the two
        # subs, ACT exps/squares/copies, DVE the coefficient math.
        F32R = mybir.dt.float32r
        psp_cm = tc.tile_pool(name="psp", bufs=1, space="PSUM")
        psp = psp_cm.__enter__()
        CW = NY * L // 2   # 1024 exp cols per chunk
        CWM = W // 2       # 1016 coef cols per chunk
        c1h = {}
        rm1h = {}
        for a in range(NX if upto != "A" else 0):
            for cb in range(2):
                kps = psp.tile([L, CW], F32, tag="kps", bufs=2)  # 2 banks
                kpsh = psp.tile([M, CW], F32, tag="kpsh", bufs=2)
                for blk in range(2):
                    rsl = slice(cb * CW + blk * 512, cb * CW + (blk + 1) * 512)
                    osl = slice(blk * 512, (blk + 1) * 512)
                    nc.tensor.matmul(
                        kps[:, osl],
                        LH[:, a * L : (a + 1) * L],
                        RH[:, rsl],
                    )
                    nc.tensor.matmul(
                        kpsh[:, osl],
                        LH[:, a * L + 1 : (a + 1) * L],
                        RH[:, rsl],
                    )
                ke = workp.tile([L, CW], F32, tag="ke", bufs=2)
                nc.scalar.activation(ke, kps, AF.Exp)
                kesh = workp.tile([M, CW], F32, tag="kesh", bufs=2)
                nc.scalar.activation(kesh, kpsh, AF.Exp)
                ed = workp.tile([M, CW], F32, tag="ed", bufs=2)
                nc.vector.tensor_sub(ed, kesh, ke[0:M, :])
                edv = ed.rearrange("p (b j) -> p b j", b=NY // 2)
                diff = workp.tile([M, CWM], F32, tag="diff", bufs=2)
                nc.vector.tensor_sub(
                    diff.rearrange("p (b j) -> p b j", b=NY // 2),
                    edv[:, :, 1:L], edv[:, :, 0:M],
                )
                # qb = (diff/sqrt(192))^2 ; c1m1 = diff/8 + qb (fp16)
                # rec = 1/(1+c1m1) ; n = qb + c1m1 ; rm1 = -n*rec (fp16)
                qb = workp.tile([M, CWM], F32, tag="qb", bufs=2)
                nc.scalar.activation(
                    qb, diff, AF.Square, scale=1.0 / np.sqrt(192.0)
                )
                c1 = workp.tile([M, CWM], F16, tag="c1m1", bufs=4)
                nc.vector.scalar_tensor_tensor(
                    c1, diff, 0.125, qb, AL.mult, AL.add
                )
                c1f = workp.tile([M, CWM], F32, tag="c1f", bufs=2)
                nc.scalar.activation(c1f, c1, AF.Copy, bias=1.0)
                rec = workp.tile([M, CWM], F32, tag="rec", bufs=2)
                nc.vector.reciprocal(rec, c1f)
                n = workp.tile([M, CWM], F32, tag="n", bufs=2)
                nc.gpsimd.tensor_add(n, qb, c1)
                rm = workp.tile([M, CWM], F16, tag="rm1", bufs=4)
                nc.vector.scalar_tensor_tensor(
                    rm, n, -1.0, rec, AL.mult, AL.mult
                )
                c1h[(a, cb)] = c1
                rm1h[(a, cb)] = rm

        # ---- Phase D: DRAM bounce into skewed pair-major layout ----
        # CHC/CHR [128, SLOTS*32] fp16; partition 32g+16a+b; group g covers
        # coarse cols 32g..32g+31 (g=3: 31 + zero pad); slot k holds coarse
        # row k-g. Stores are contiguous row-major (127 descs of 4KB); loads
        # gather the per-group column slices.
        if upto == "C":
            nc.sync.dma_start(
                out=out_t[:, :], in_=c1h[(1, 1)][0:NP, 0:2].bitcast(F32)
            )
        if upto in ("A", "C"):
            nc.finalize_after_pools = True  # sentinel unused; early build
        else:
            dramp = ctx.enter_context(
                tc.tile_pool(name="dramp", bufs=1, space="DRAM")
            )
            # lo/hi split by slot so phase E can start while hi loads/expands
            SLO = 66  # slots 0..65 in lo tiles, 66..SLOTS-1 in hi
            CHt = {}
            for nm2, part, nsl in (("c", "lo", SLO), ("c", "hi", SLOTS - SLO),
                                   ("r", "lo", SLO), ("r", "hi", SLOTS - SLO)):
                t = chp.tile([4 * NP, nsl * 32], F16, tag=f"CH{nm2}_{part}")
                nc.vector.memset(t, 0.0)
                CHt[(nm2, part)] = t
            di = 0
            drt = {}
            for a in range(NX):
                for nm, arrs in (("c1", c1h), ("rm", rm1h)):
                    d = dramp.tile([M, W], F16, tag=f"d_{nm}{a}")
                    for cb in range(2):
                        rings[di % 3].dma_start(
                            out=d[:, cb * CWM : (cb + 1) * CWM],
                            in_=arrs[(a, cb)][0:M, :],
                        )
                        di += 1
                    drt[(nm, a)] = d
            for part in ("lo", "hi"):
                for g in range(4):
                    w = 32 if g < 3 else 31
                    # slot k holds coarse row k-g; lo covers slots g..SLO-1
                    if part == "lo":
                        r0, r1 = 0, SLO - g        # coarse rows
                        base = 0
                    else:
                        r0, r1 = SLO - g, M
                        base = SLO
                    for a in range(NX):
                        for nm2, nm in (("c", "c1"), ("r", "rm")):
                            CH = CHt[(nm2, part)]
                            d = drt[(nm, a)]
                            dst = CH[
                                32 * g + 16 * a : 32 * g + 16 * a + 16,
                                (g + r0 - base) * 32 : (g + r1 - base) * 32,
                            ].rearrange("p (r c) -> p r c", c=32)[:, :, 0:w]
                            src = d.rearrange("r (q c) -> q r c", q=NY)[
                                :, r0:r1, 32 * g : 32 * g + w
                            ]
                            rings[di % 3].dma_start(out=dst, in_=src)
                            di += 1

        # ---- Phase E: 4-group wavefront ----
        psp_cm.__exit__(None, None, None)
        if upto == "D":
            nc.sync.dma_start(
                out=out_t[:, :], in_=CHt[("c", "hi")][0:NP, 0:2].bitcast(F32)
            )
        if upto in ("A", "C", "D"):
            nstep = 0
        else:
            psE = ctx.enter_context(
                tc.tile_pool(name="psE", bufs=1, space="PSUM")
            )
            # bulk coefficient expansion: fp16 (c-1) -> fp32 c, repeat2
            # lo/hi expansion tiles; hi expands while phase E runs on lo
            pc1_lo = chp.tile([4 * NP, SLO * 64], F32, tag="PC1_lo")
            pc1_hi = chp.tile([4 * NP, (SLOTS - SLO) * 64], F32, tag="PC1_hi")
            pr_lo = chp.tile([4 * NP, SLO * 64], F32, tag="PR_lo")
            pr_hi = chp.tile([4 * NP, (SLOTS - SLO) * 64], F32, tag="PR_hi")
            PC1 = {"lo": pc1_lo, "hi": pc1_hi}
            PR = {"lo": pr_lo, "hi": pr_hi}
            for part, bounds in (("lo", (0, 8, 24, SLO)),
                                 ("hi", (0, SLOTS - SLO))):
                for ci in range(len(bounds) - 1):
                    s0, s1 = bounds[ci], bounds[ci + 1]
                    nc.scalar.activation(
                        PC1[part][:, s0 * 64 : s1 * 64].rearrange(
                            "p (a b) -> p a b", b=2),
                        _rep2(CHt[("c", part)][:, s0 * 32 : s1 * 32]),
                        AF.Copy, bias=1.0,
                    )
                    nc.scalar.activation(
                        PR[part][:, s0 * 64 : s1 * 64].rearrange(
                            "p (a b) -> p a b", b=2),
                        _rep2(CHt[("r", part)][:, s0 * 32 : s1 * 32]),
                        AF.Copy, bias=1.0,
                    )
            GA = constp.tile([4 * NP, 66], F32)
            GB = constp.tile([4 * NP, 66], F32)
            nc.vector.memset(GA, 1.0)
            nc.vector.memset(GB, 1.0)
            B3 = []
            for k in range(3):
                b = psE.tile([4 * NP, 1], F32, tag=f"b3_{k}")
                # initialize boundary buffers to 1.0 via the ones matmul
                nc.tensor.matmul(b, e0row, one1, start=True, stop=False)
                nc.tensor.matmul(
                    b, shp, GA[:, 64:65], start=False, stop=True
                )
                B3.append(b)

        for T in range(1, nstep + 1):
            slot = (T - 1) // 2
            part = "lo" if slot < SLO else "hi"
            sl = slot if part == "lo" else slot - SLO
            exp_c = PC1[part][:, sl * 64 : (sl + 1) * 64]
            exp_r = PR[part][:, sl * 64 : (sl + 1) * 64]
            cur, prv = (GA, GB) if T % 2 == 1 else (GB, GA)
            kb = T % 3
            kread = (T + 1) % 3  # holds boundary produced at step T-2
            # boundary -> cur col 0 (k0 for next step's mul; scan initial)
            nc.vector.tensor_copy(cur[:, 0:1], B3[kread][:, 0:1])
            t64 = ep.tile([4 * NP, 64], F32, tag="t64")
            m = ep.tile([4 * NP, 64], F32, tag="m")
            nc.vector.tensor_mul(t64, exp_r, prv[:, 0:64])
            nc.vector.tensor_sub(m, prv[:, 1:65], t64)
            nc.vector.tensor_tensor_scan(
                cur[:, 1:65], m, exp_c, cur[:, 0:1], AL.add, AL.mult
            )
            # boundary out via PE shift: B3[kb] = shp^T. @ cur[:,64] + e0
            nc.tensor.matmul(B3[kb], e0row, one1, start=True, stop=False)
            nc.tensor.matmul(B3[kb], shp, cur[:, 64:65], start=False, stop=True)

        if nstep > 0:
            final = GB if nstep % 2 == 0 else GA
            nc.sync.dma_start(out=out_t[:, :], in_=final[96:128, 62:63])

    nc.finalize()
    return nc


_CACHE = {}


def _get_nc():
    if "nc" not in _CACHE:
        _CACHE["nc"] = _build()
    return _CACHE["nc"]


def run(xs, ys, trace=False):
    xs = np.ascontiguousarray(np.asarray(xs), dtype=np.float32)
    ys = np.ascontiguousarray(np.asarray(ys), dtype=np.float32)
    assert xs.shape == (16, L, D) and ys.shape == (16, L, D)
    nc = _get_nc()
    idn = np.eye(L, dtype=np.float32)
    shf = np.eye(L, k=-1, dtype=np.float32)
    shp = np.eye(L, k=32, dtype=np.float32)
    in_maps = []
    for c in range(N_CORES):
        in_maps.append(
            {
                "xs": xs[2 * c : 2 * c + 2].reshape(NX * L, D).copy(),
                "ys": ys.reshape(NY * L, D).copy(),
                "idn": idn,
                "shf": shf,
                "shp": shp,
            }
        )
    try:
        res = run_bass_kernel_spmd(nc, in_maps, list(range(N_CORES)), trace=trace)
    except ModuleNotFoundError:
        res = run_bass_kernel_spmd(nc, in_maps, list(range(N_CORES)), trace=False)
    rows = [res.results[c]["out"].reshape(NX, NY) for c in range(N_CORES)]
    out = np.concatenate(rows, axis=0)
    return out, res


def kernel(xs, ys):
    out, _ = run(xs, ys)
    return out


# revision 5
# speedup vs baseline: 1.0392x; 1.0004x over previous
"""Signature-kernel Gram matrix on 8 NeuronCores.

Math per pair (x (128,8), y (128,8)):
  K = exp(x@y.T - 0.5|x|^2 - 0.5|y|^2)           RBF gram, sigma=1
  diff = second mixed finite difference of K      (127,127)
  Goursat PDE on the dyadic-refined fine grid G (255,255), G[0,:]=G[:,0]=1,
    G[i,j] = c1*(G[i-1,j]+G[i,j-1]) - c2*G[i-1,j-1]
    with inc = diff/4 constant on 2x2 fine blocks,
    c1 = 1 + diff/8 + diff^2/192, c2 = 1 - diff^2/192
  answer = G[254,254]

Structure (one core = 32 pairs = 2 local xs x 16 ys):
  A: batched loads, PE transposes, stacked matmul operands
     LH[10,256] = [x^T; -|x|^2/2; 1], RH[10,2048] = [y^T; 1; -|y|^2/2]
     (norms via ACT square + DVE reduce + one PE transpose).
  B: per (x-row a, 8-pair chunk): one K=10 matmul for the exponent AND
     one for the row-shifted exponent (lhsT column-offset by 1), ACT exp;
     diff = col-diff of (Ksh - K) on DVE.
  C: c1-1 and r-1 = c2/c1-1 as fp16 (values are tiny, so fp16 on the
     DELTAS keeps ~1e-5 abs precision); reciprocal on DVE.
  D: flatten to pair-major via DRAM bounce: contiguous row-major stores
     (127 descriptors x 4KB), then strided per-column-block loads into
     pre-skewed CH tiles (lo/hi slot split so phase E starts early).
     Layout: partition 32g+16a+b owns pair (a,b), column block g; slot k
     of the CH array holds coarse row k-g (skew 2 rows per group).
  E: bulk-expand c1/r rows (repeat2, +1.0 bias, fp32) once on ACT, then
     254+6 wavefront steps. Step T runs block-group g at fine row T-2g:
     DVE: copy boundary->cur[:,0], t=r*k0, m=k1-t,
          scan state=(m+state)*c1 over all 128 partitions (scan initial
          = cur[:,0]); PE shifts scan-end columns +32 partitions into a
          rotating PSUM buffer (plus an e0 matmul for group 0's 1.0).
  Output: group 3 partitions, local column 62 = G[254,254] per pair.

Sharding: data-parallel over batch_x: core c owns x rows {2c, 2c+1} x all
16 ys. Host gathers the (16,16) output.
"""

import numpy as np
from contextlib import ExitStack

import concourse.bass as bass
import concourse.bacc as bacc
import concourse.tile as tile
from concourse import mybir
from concourse.bass_utils import run_bass_kernel_spmd

F32 = mybir.dt.float32
F16 = mybir.dt.float16
AL = mybir.AluOpType
AF = mybir.ActivationFunctionType

N_CORES = 8
L = 128          # sequence length
D = 8            # feature dim
NY = 16          # ys per core
NX = 2           # xs per core
NP = NX * NY     # 32 pairs per core
M = L - 1        # 127 coarse grid
G = 2 * M        # 254 fine grid (dyadic order 1)
NSEQ = NX + NY   # 18
NSTEP = G + 6    # wavefront steps (4 groups, skew 2)
SLOTS = NSTEP // 2 + 1  # coarse slots incl skew pad


def _rep2(ap):
    """[P, n] view -> [P, n, 2] with zero-stride inner dim."""
    return bass.AP(tensor=ap.tensor, offset=ap.offset,
                   ap=[ap.ap[0], ap.ap[1], [0, 2]])


def _build(upto="full", nstep=NSTEP):
    nc = bacc.Bacc()
    xs_t = nc.dram_tensor("xs", [NX * L, D], F32, kind="ExternalInput")
    ys_t = nc.dram_tensor("ys", [NY * L, D], F32, kind="ExternalInput")
    idn_t = nc.dram_tensor("idn", [L, L], F32, kind="ExternalInput")
    shf_t = nc.dram_tensor("shf", [L, L], F32, kind="ExternalInput")
    shp_t = nc.dram_tensor("shp", [L, L], F32, kind="ExternalInput")
    out_t = nc.dram_tensor("out", [NP, 1], F32, kind="ExternalOutput")

    rings = None  # set after pools

    with ExitStack() as ctx:
        tc = ctx.enter_context(tile.TileContext(nc))
        constp = ctx.enter_context(tc.tile_pool(name="constp", bufs=1))
        iop = ctx.enter_context(tc.tile_pool(name="iop", bufs=3))
        workp = ctx.enter_context(tc.tile_pool(name="workp", bufs=2))
        chp = ctx.enter_context(tc.tile_pool(name="chp", bufs=1))
        ep = ctx.enter_context(tc.tile_pool(name="ep", bufs=2))

        rings = [nc.sync, nc.scalar, nc.gpsimd]

        # ---- Phase A ----
        idn_s = iop.tile([L, L], F32, tag="idn_s")
        nc.sync.dma_start(out=idn_s, in_=idn_t[:, :])
        idn = constp.tile([L, L], F32)
        nc.vector.tensor_copy(idn, idn_s)
        shp_s = iop.tile([L, L], F32, tag="shp_s")
        nc.gpsimd.dma_start(out=shp_s, in_=shp_t[:, :])
        shp = constp.tile([L, L], F32)
        nc.vector.tensor_copy(shp, shp_s)
        ones8 = constp.tile([D, 1], F32)
        nc.vector.memset(ones8, 1.0)
        one1 = constp.tile([1, 1], F32)
        nc.vector.memset(one1, 1.0)
        e0row = constp.tile([1, L], F32)
        nc.vector.memset(e0row, 0.0)
        nc.vector.memset(e0row[:, 0:32], 1.0)

        # LH rows: 0-7 x^T, 8 = -0.5|x|^2, 9 = ones   (cols: a*L..)
        # RH rows: 0-7 y^T, 8 = ones, 9 = -0.5|y|^2   (cols: b*L..)
        LH = constp.tile([D + 2, NX * L], F32)
        RH = constp.tile([D + 2, NY * L], F32)
        ones2k = constp.tile([1, NY * L], F32)
        nc.vector.memset(ones2k, 1.0)
        # rows 8/9 are written via DMA (compute ops must start at partition 0)
        nc.sync.dma_start(out=LH[D + 1 : D + 2, :], in_=ones2k[:, 0 : NX * L])
        nc.scalar.dma_start(out=RH[D : D + 1, :], in_=ones2k[:, :])

        with tc.tile_pool(name="psA", bufs=1, space="PSUM") as psA, \
             tc.tile_pool(name="awork", bufs=1) as awork:
            # batched sequence loads: [i-partition, (seq, feat)]
            xr_s = awork.tile([L, NX * D], F32, tag="xr_s")
            nc.scalar.dma_start(
                out=xr_s, in_=xs_t.rearrange("(a i) k -> i a k", a=NX)
            )
            xr = awork.tile([L, NX * D], F32, tag="xr")
            nc.vector.tensor_copy(xr, xr_s)
            yr_s = awork.tile([L, NY * D], F32, tag="yr_s")
            nc.sync.dma_start(
                out=yr_s, in_=ys_t.rearrange("(b i) k -> i b k", b=NY)
            )
            yr = awork.tile([L, NY * D], F32, tag="yr")
            nc.vector.tensor_copy(yr, yr_s)
            # transposes: 4 per PSUM bank tile, one ACT copy per bank
            psx = psA.tile([D, NX * L], F32, tag="psx", bufs=1)
            for a in range(NX):
                nc.tensor.transpose(
                    psx[:, a * L : (a + 1) * L], xr[:, a * D : (a + 1) * D], idn
                )
            nc.scalar.activation(LH[0:D, :], psx, AF.Copy)
            for yb in range(4):
                psy = psA.tile([D, 4 * L], F32, tag="psy", bufs=2)
                for j in range(4):
                    b = 4 * yb + j
                    nc.tensor.transpose(
                        psy[:, j * L : (j + 1) * L],
                        yr[:, b * D : (b + 1) * D], idn,
                    )
                nc.scalar.activation(
                    RH[0:D, yb * 512 : (yb + 1) * 512], psy, AF.Copy
                )

            # norms: square+reduce in sequence-index layout, one transpose,
            # then DMA rows into LH/RH
            sq = awork.tile([L, (NX + NY) * D], F32, tag="sq")
            nc.scalar.square(sq[:, 0 : NY * D], yr)
            nc.scalar.square(sq[:, NY * D :], xr)
            nr = awork.tile([L, NX + NY], F32, tag="nr")
            nc.vector.tensor_reduce(
                nr, sq.rearrange("p (s k) -> p s k", k=D),
                mybir.AxisListType.X, AL.add,
            )
            nrp = psA.tile([NX + NY, L], F32, tag="nrp", bufs=1)
            nc.tensor.transpose(nrp, nr, idn)
            nrt = awork.tile([NX + NY, L], F32, tag="nrt")
            nc.scalar.activation(nrt, nrp, AF.Copy, scale=-0.5)
            nc.sync.dma_start(out=RH[D + 1 : D + 2, :], in_=nrt[0:NY, :])
            nc.gpsimd.dma_start(out=LH[D : D + 1, :], in_=nrt[NY : NY + NX, :])

        # coefficient staging tiles (per half a): [127p, 16 pairs * 127]
        W = NY * M  # 2032
        c1h = [None, None]
        rm1h = [None, None]

        if upto == "A":
            nc.sync.dma_start(out=out_t[0:2, :], in_=LH[0:2, 0:1])

        # ---- Phases B + C, two 8-pair chunks per half, pipelined ----
        # Exponent AND row-shifted exponent via two matmul sets (f32r);
        # diff = col-diff of (Ksh - K). Engine balance: Pool does the two
        # subs, ACT exps/squares/copies, DVE the coefficient math.
        F32R = mybir.dt.float32r
        psp_cm = tc.tile_pool(name="psp", bufs=1, space="PSUM")
        psp = psp_cm.__enter__()
        CW = NY * L // 2   # 1024 exp cols per chunk
        CWM = W // 2       # 1016 coef cols per chunk
        c1h = {}
        rm1h = {}
        for a in range(NX if upto != "A" else 0):
            for cb in range(2):
                kps = psp.tile([L, CW], F32, tag="kps", bufs=2)  # 2 banks
                kpsh = psp.tile([M, CW], F32, tag="kpsh", bufs=2)
                for blk in range(2):
                    rsl = slice(cb * CW + blk * 512, cb * CW + (blk + 1) * 512)
                    osl = slice(blk * 512, (blk + 1) * 512)
                    nc.tensor.matmul(
                        kps[:, osl],
                        LH[:, a * L : (a + 1) * L],
                        RH[:, rsl],
                    )
                    nc.tensor.matmul(
                        kpsh[:, osl],
                        LH[:, a * L + 1 : (a + 1) * L],
                        RH[:, rsl],
                    )
                ke = workp.tile([L, CW], F32, tag="ke", bufs=3)
                nc.scalar.activation(ke, kps, AF.Exp)
                kesh = workp.tile([M, CW], F32, tag="kesh", bufs=2)
                nc.scalar.activation(kesh, kpsh, AF.Exp)
                ed = workp.tile([M, CW], F32, tag="ed", bufs=2)
                nc.vector.tensor_sub(ed, kesh, ke[0:M, :])
                edv = ed.rearrange("p (b j) -> p b j", b=NY // 2)
                diff = workp.tile([M, CWM], F32, tag="diff", bufs=2)
                nc.vector.tensor_sub(
                    diff.rearrange("p (b j) -> p b j", b=NY // 2),
                    edv[:, :, 1:L], edv[:, :, 0:M],
                )
                # qb = (diff/sqrt(192))^2 ; c1m1 = diff/8 + qb (fp16)
                # rec = 1/(1+c1m1) ; n = qb + c1m1 ; rm1 = -n*rec (fp16)
                qb = workp.tile([M, CWM], F32, tag="qb", bufs=2)
                nc.scalar.activation(
                    qb, diff, AF.Square, scale=1.0 / np.sqrt(192.0)
                )
                c1 = workp.tile([M, CWM], F16, tag="c1m1", bufs=4)
                nc.vector.scalar_tensor_tensor(
                    c1, diff, 0.125, qb, AL.mult, AL.add
                )
                c1f = workp.tile([M, CWM], F32, tag="c1f", bufs=2)
                nc.scalar.activation(c1f, c1, AF.Copy, bias=1.0)
                rec = workp.tile([M, CWM], F32, tag="rec", bufs=2)
                nc.vector.reciprocal(rec, c1f)
                n = workp.tile([M, CWM], F32, tag="n", bufs=2)
                nc.gpsimd.tensor_add(n, qb, c1)
                rm = workp.tile([M, CWM], F16, tag="rm1", bufs=4)
                nc.vector.scalar_tensor_tensor(
                    rm, n, -1.0, rec, AL.mult, AL.mult
                )
                c1h[(a, cb)] = c1
                rm1h[(a, cb)] = rm

        # ---- Phase D: DRAM bounce into skewed pair-major layout ----
        # CHC/CHR [128, SLOTS*32] fp16; partition 32g+16a+b; group g covers
        # coarse cols 32g..32g+31 (g=3: 31 + zero pad); slot k holds coarse
        # row k-g. Stores are contiguous row-major (127 descs of 4KB); loads
        # gather the per-group column slices.
        if upto == "C":
            nc.sync.dma_start(
                out=out_t[:, :], in_=c1h[(1, 1)][0:NP, 0:2].bitcast(F32)
            )
        if upto in ("A", "C"):
            nc.finalize_after_pools = True  # sentinel unused; early build
        else:
            dramp = ctx.enter_context(
                tc.tile_pool(name="dramp", bufs=1, space="DRAM")
            )
            # lo/hi split by slot so phase E can start while hi loads/expands
            SLO = 66  # slots 0..65 in lo tiles, 66..SLOTS-1 in hi
            CHt = {}
            for nm2, part, nsl in (("c", "lo", SLO), ("c", "hi", SLOTS - SLO),
                                   ("r", "lo", SLO), ("r", "hi", SLOTS - SLO)):
                t = chp.tile([4 * NP, nsl * 32], F16, tag=f"CH{nm2}_{part}")
                nc.vector.memset(t, 0.0)
                CHt[(nm2, part)] = t
            di = 0
            drt = {}
            for a in range(NX):
                for nm, arrs in (("c1", c1h), ("rm", rm1h)):
                    d = dramp.tile([M, W], F16, tag=f"d_{nm}{a}")
                    for cb in range(2):
                        rings[di % 3].dma_start(
                            out=d[:, cb * CWM : (cb + 1) * CWM],
                            in_=arrs[(a, cb)][0:M, :],
                        )
                        di += 1
                    drt[(nm, a)] = d
            for part in ("lo", "hi"):
                for g in range(4):
                    w = 32 if g < 3 else 31
                    # slot k holds coarse row k-g; lo covers slots g..SLO-1
                    if part == "lo":
                        r0, r1 = 0, SLO - g        # coarse rows
                        base = 0
                    else:
                        r0, r1 = SLO - g, M
                        base = SLO
                    for a in range(NX):
                        for nm2, nm in (("c", "c1"), ("r", "rm")):
                            CH = CHt[(nm2, part)]
                            d = drt[(nm, a)]
                            dst = CH[
                                32 * g + 16 * a : 32 * g + 16 * a + 16,
                                (g + r0 - base) * 32 : (g + r1 - base) * 32,
                            ].rearrange("p (r c) -> p r c", c=32)[:, :, 0:w]
                            src = d.rearrange("r (q c) -> q r c", q=NY)[
                                :, r0:r1, 32 * g : 32 * g + w
                            ]
                            rings[di % 3].dma_start(out=dst, in_=src)
                            di += 1

        # ---- Phase E: 4-group wavefront ----
        psp_cm.__exit__(None, None, None)
        if upto == "D":
            nc.sync.dma_start(
                out=out_t[:, :], in_=CHt[("c", "hi")][0:NP, 0:2].bitcast(F32)
            )
        if upto in ("A", "C", "D"):
            nstep = 0
        else:
            psE = ctx.enter_context(
                tc.tile_pool(name="psE", bufs=1, space="PSUM")
            )
            # bulk coefficient expansion: fp16 (c-1) -> fp32 c, repeat2
            # lo/hi expansion tiles; hi expands while phase E runs on lo
            pc1_lo = chp.tile([4 * NP, SLO * 64], F32, tag="PC1_lo")
            pc1_hi = chp.tile([4 * NP, (SLOTS - SLO) * 64], F32, tag="PC1_hi")
            pr_lo = chp.tile([4 * NP, SLO * 64], F32, tag="PR_lo")
            pr_hi = chp.tile([4 * NP, (SLOTS - SLO) * 64], F32, tag="PR_hi")
            PC1 = {"lo": pc1_lo, "hi": pc1_hi}
            PR = {"lo": pr_lo, "hi": pr_hi}
            for part, bounds in (("lo", (0, 8, 24, SLO)),
                                 ("hi", (0, SLOTS - SLO))):
                for ci in range(len(bounds) - 1):
                    s0, s1 = bounds[ci], bounds[ci + 1]
                    nc.scalar.activation(
                        PC1[part][:, s0 * 64 : s1 * 64].rearrange(
                            "p (a b) -> p a b", b=2),
                        _rep2(CHt[("c", part)][:, s0 * 32 : s1 * 32]),
                        AF.Copy, bias=1.0,
                    )
                    nc.scalar.activation(
                        PR[part][:, s0 * 64 : s1 * 64].rearrange(
                            "p (a b) -> p a b", b=2),
                        _rep2(CHt[("r", part)][:, s0 * 32 : s1 * 32]),
                        AF.Copy, bias=1.0,
                    )
            GA = constp.tile([4 * NP, 66], F32)
            GB = constp.tile([4 * NP, 66], F32)
            nc.vector.memset(GA, 1.0)
            nc.vector.memset(GB, 1.0)
            B3 = []
            for k in range(3):
                b = psE.tile([4 * NP, 1], F32, tag=f"b3_{k}")
                # initialize boundary buffers to 1.0 via the ones matmul
                nc.tensor.matmul(b, e0row, one1, start=True, stop=False)
                nc.tensor.matmul(
                    b, shp, GA[:, 64:65], start=False, stop=True
                )
                B3.append(b)

        for T in range(1, nstep + 1):
            slot = (T - 1) // 2
            part = "lo" if slot < SLO else "hi"
            sl = slot if part == "lo" else slot - SLO
            exp_c = PC1[part][:, sl * 64 : (sl + 1) * 64]
            exp_r = PR[part][:, sl * 64 : (sl + 1) * 64]
            cur, prv = (GA, GB) if T % 2 == 1 else (GB, GA)
            kb = T % 3
            kread = (T + 1) % 3  # holds boundary produced at step T-2
            # boundary -> cur col 0 (k0 for next step's mul; scan initial)
            nc.vector.tensor_copy(cur[:, 0:1], B3[kread][:, 0:1])
            t64 = ep.tile([4 * NP, 64], F32, tag="t64")
            m = ep.tile([4 * NP, 64], F32, tag="m")
            nc.vector.tensor_mul(t64, exp_r, prv[:, 0:64])
            nc.vector.tensor_sub(m, prv[:, 1:65], t64)
            nc.vector.tensor_tensor_scan(
                cur[:, 1:65], m, exp_c, cur[:, 0:1], AL.add, AL.mult
            )
            # boundary out via PE shift: B3[kb] = shp^T. @ cur[:,64] + e0
            nc.tensor.matmul(B3[kb], e0row, one1, start=True, stop=False)
            nc.tensor.matmul(B3[kb], shp, cur[:, 64:65], start=False, stop=True)

        if nstep > 0:
            final = GB if nstep % 2 == 0 else GA
            nc.sync.dma_start(out=out_t[:, :], in_=final[96:128, 62:63])

    nc.finalize()
    return nc


_CACHE = {}


def _get_nc():
    if "nc" not in _CACHE:
        _CACHE["nc"] = _build()
    return _CACHE["nc"]


def run(xs, ys, trace=False):
    xs = np.ascontiguousarray(np.asarray(xs), dtype=np.float32)
    ys = np.ascontiguousarray(np.asarray(ys), dtype=np.float32)
    assert xs.shape == (16, L, D) and ys.shape == (16, L, D)
    nc = _get_nc()
    idn = np.eye(L, dtype=np.float32)
    shf = np.eye(L, k=-1, dtype=np.float32)
    shp = np.eye(L, k=32, dtype=np.float32)
    in_maps = []
    for c in range(N_CORES):
        in_maps.append(
            {
                "xs": xs[2 * c : 2 * c + 2].reshape(NX * L, D).copy(),
                "ys": ys.reshape(NY * L, D).copy(),
                "idn": idn,
                "shf": shf,
                "shp": shp,
            }
        )
    try:
        res = run_bass_kernel_spmd(nc, in_maps, list(range(N_CORES)), trace=trace)
    except ModuleNotFoundError:
        res = run_bass_kernel_spmd(nc, in_maps, list(range(N_CORES)), trace=False)
    rows = [res.results[c]["out"].reshape(NX, NY) for c in range(N_CORES)]
    out = np.concatenate(rows, axis=0)
    return out, res


def kernel(xs, ys):
    out, _ = run(xs, ys)
    return out


# revision 6
# speedup vs baseline: 1.0406x; 1.0014x over previous
"""Signature-kernel Gram matrix on 8 NeuronCores.

Math per pair (x (128,8), y (128,8)):
  K = exp(x@y.T - 0.5|x|^2 - 0.5|y|^2)           RBF gram, sigma=1
  diff = second mixed finite difference of K      (127,127)
  Goursat PDE on the dyadic-refined fine grid G (255,255), G[0,:]=G[:,0]=1,
    G[i,j] = c1*(G[i-1,j]+G[i,j-1]) - c2*G[i-1,j-1]
    with inc = diff/4 constant on 2x2 fine blocks,
    c1 = 1 + diff/8 + diff^2/192, c2 = 1 - diff^2/192
  answer = G[254,254]

Structure (one core = 32 pairs = 2 local xs x 16 ys):
  A: batched loads, PE transposes, stacked matmul operands
     LH[10,256] = [x^T; -|x|^2/2; 1], RH[10,2048] = [y^T; 1; -|y|^2/2]
     (norms via ACT square + DVE reduce + one PE transpose).
  B: per (x-row a, 8-pair chunk): one K=10 matmul for the exponent AND
     one for the row-shifted exponent (lhsT column-offset by 1), ACT exp;
     diff = col-diff of (Ksh - K) on DVE.
  C: c1-1 and r-1 = c2/c1-1 as fp16 (values are tiny, so fp16 on the
     DELTAS keeps ~1e-5 abs precision); reciprocal on DVE.
  D: flatten to pair-major via DRAM bounce: contiguous row-major stores
     (127 descriptors x 4KB), then strided per-column-block loads into
     pre-skewed CH tiles (lo/hi slot split so phase E starts early).
     Layout: partition 32g+16a+b owns pair (a,b), column block g; slot k
     of the CH array holds coarse row k-g (skew 2 rows per group).
  E: bulk-expand c1/r rows (repeat2, +1.0 bias, fp32) once on ACT, then
     254+6 wavefront steps. Step T runs block-group g at fine row T-2g:
     DVE: copy boundary->cur[:,0], t=r*k0, m=k1-t,
          scan state=(m+state)*c1 over all 128 partitions (scan initial
          = cur[:,0]); PE shifts scan-end columns +32 partitions into a
          rotating PSUM buffer (plus an e0 matmul for group 0's 1.0).
  Output: group 3 partitions, local column 62 = G[254,254] per pair.

Sharding: data-parallel over batch_x: core c owns x rows {2c, 2c+1} x all
16 ys. Host gathers the (16,16) output.
"""

import numpy as np
from contextlib import ExitStack

import concourse.bass as bass
import concourse.bacc as bacc
import concourse.tile as tile
from concourse import mybir
from concourse.bass_utils import run_bass_kernel_spmd

F32 = mybir.dt.float32
F16 = mybir.dt.float16
AL = mybir.AluOpType
AF = mybir.ActivationFunctionType

N_CORES = 8
L = 128          # sequence length
D = 8            # feature dim
NY = 16          # ys per core
NX = 2           # xs per core
NP = NX * NY     # 32 pairs per core
M = L - 1        # 127 coarse grid
G = 2 * M        # 254 fine grid (dyadic order 1)
NSEQ = NX + NY   # 18
NSTEP = G + 6    # wavefront steps (4 groups, skew 2)
SLOTS = NSTEP // 2 + 1  # coarse slots incl skew pad


def _rep2(ap):
    """[P, n] view -> [P, n, 2] with zero-stride inner dim."""
    return bass.AP(tensor=ap.tensor, offset=ap.offset,
                   ap=[ap.ap[0], ap.ap[1], [0, 2]])


def _build(upto="full", nstep=NSTEP):
    nc = bacc.Bacc()
    xs_t = nc.dram_tensor("xs", [NX * L, D], F32, kind="ExternalInput")
    ys_t = nc.dram_tensor("ys", [NY * L, D], F32, kind="ExternalInput")
    idn_t = nc.dram_tensor("idn", [L, L], F32, kind="ExternalInput")
    shf_t = nc.dram_tensor("shf", [L, L], F32, kind="ExternalInput")
    shp_t = nc.dram_tensor("shp", [L, L], F32, kind="ExternalInput")
    out_t = nc.dram_tensor("out", [NP, 1], F32, kind="ExternalOutput")

    rings = None  # set after pools

    with ExitStack() as ctx:
        tc = ctx.enter_context(tile.TileContext(nc))
        constp = ctx.enter_context(tc.tile_pool(name="constp", bufs=1))
        iop = ctx.enter_context(tc.tile_pool(name="iop", bufs=3))
        workp = ctx.enter_context(tc.tile_pool(name="workp", bufs=2))
        chp = ctx.enter_context(tc.tile_pool(name="chp", bufs=1))
        ep = ctx.enter_context(tc.tile_pool(name="ep", bufs=2))

        rings = [nc.sync, nc.scalar, nc.gpsimd]

        # ---- Phase A ----
        idn_s = iop.tile([L, L], F32, tag="idn_s")
        nc.sync.dma_start(out=idn_s, in_=idn_t[:, :])
        idn = constp.tile([L, L], F32)
        nc.vector.tensor_copy(idn, idn_s)
        shp_s = iop.tile([L, L], F32, tag="shp_s")
        nc.gpsimd.dma_start(out=shp_s, in_=shp_t[:, :])
        shp = constp.tile([L, L], F32)
        nc.vector.tensor_copy(shp, shp_s)
        ones8 = constp.tile([D, 1], F32)
        nc.vector.memset(ones8, 1.0)
        one1 = constp.tile([1, 1], F32)
        nc.vector.memset(one1, 1.0)
        e0row = constp.tile([1, L], F32)
        nc.vector.memset(e0row, 0.0)
        nc.vector.memset(e0row[:, 0:32], 1.0)

        # LH rows: 0-7 x^T, 8 = -0.5|x|^2, 9 = ones   (cols: a*L..)
        # RH rows: 0-7 y^T, 8 = ones, 9 = -0.5|y|^2   (cols: b*L..)
        LH = constp.tile([D + 2, NX * L], F32)
        RH = constp.tile([D + 2, NY * L], F32)
        ones2k = constp.tile([1, NY * L], F32)
        nc.vector.memset(ones2k, 1.0)
        # rows 8/9 are written via DMA (compute ops must start at partition 0)
        nc.sync.dma_start(out=LH[D + 1 : D + 2, :], in_=ones2k[:, 0 : NX * L])
        nc.scalar.dma_start(out=RH[D : D + 1, :], in_=ones2k[:, :])

        with tc.tile_pool(name="psA", bufs=1, space="PSUM") as psA, \
             tc.tile_pool(name="awork", bufs=1) as awork:
            # batched sequence loads: [i-partition, (seq, feat)]
            xr_s = awork.tile([L, NX * D], F32, tag="xr_s")
            nc.scalar.dma_start(
                out=xr_s, in_=xs_t.rearrange("(a i) k -> i a k", a=NX)
            )
            xr = awork.tile([L, NX * D], F32, tag="xr")
            nc.vector.tensor_copy(xr, xr_s)
            yr_s = awork.tile([L, NY * D], F32, tag="yr_s")
            nc.sync.dma_start(
                out=yr_s, in_=ys_t.rearrange("(b i) k -> i b k", b=NY)
            )
            yr = awork.tile([L, NY * D], F32, tag="yr")
            nc.vector.tensor_copy(yr, yr_s)
            # transposes: 4 per PSUM bank tile, one ACT copy per bank
            psx = psA.tile([D, NX * L], F32, tag="psx", bufs=1)
            for a in range(NX):
                nc.tensor.transpose(
                    psx[:, a * L : (a + 1) * L], xr[:, a * D : (a + 1) * D], idn
                )
            nc.scalar.activation(LH[0:D, :], psx, AF.Copy)
            for yb in range(4):
                psy = psA.tile([D, 4 * L], F32, tag="psy", bufs=2)
                for j in range(4):
                    b = 4 * yb + j
                    nc.tensor.transpose(
                        psy[:, j * L : (j + 1) * L],
                        yr[:, b * D : (b + 1) * D], idn,
                    )
                nc.scalar.activation(
                    RH[0:D, yb * 512 : (yb + 1) * 512], psy, AF.Copy
                )

            # norms: square+reduce in sequence-index layout, one transpose,
            # then DMA rows into LH/RH
            sq = awork.tile([L, (NX + NY) * D], F32, tag="sq")
            nc.scalar.square(sq[:, 0 : NY * D], yr)
            nc.scalar.square(sq[:, NY * D :], xr)
            nr = awork.tile([L, NX + NY], F32, tag="nr")
            nc.vector.tensor_reduce(
                nr, sq.rearrange("p (s k) -> p s k", k=D),
                mybir.AxisListType.X, AL.add,
            )
            nrp = psA.tile([NX + NY, L], F32, tag="nrp", bufs=1)
            nc.tensor.transpose(nrp, nr, idn)
            nrt = awork.tile([NX + NY, L], F32, tag="nrt")
            nc.scalar.activation(nrt, nrp, AF.Copy, scale=-0.5)
            nc.sync.dma_start(out=RH[D + 1 : D + 2, :], in_=nrt[0:NY, :])
            nc.gpsimd.dma_start(out=LH[D : D + 1, :], in_=nrt[NY : NY + NX, :])

        # coefficient staging tiles (per half a): [127p, 16 pairs * 127]
        W = NY * M  # 2032
        c1h = [None, None]
        rm1h = [None, None]

        if upto == "A":
            nc.sync.dma_start(out=out_t[0:2, :], in_=LH[0:2, 0:1])

        # ---- Phases B + C, two 8-pair chunks per half, pipelined ----
        # Exponent AND row-shifted exponent via two matmul sets (f32r);
        # diff = col-diff of (Ksh - K). Engine balance: Pool does the two
        # subs, ACT exps/squares/copies, DVE the coefficient math.
        F32R = mybir.dt.float32r
        psp_cm = tc.tile_pool(name="psp", bufs=1, space="PSUM")
        psp = psp_cm.__enter__()
        CW = NY * L // 2   # 1024 exp cols per chunk
        CWM = W // 2       # 1016 coef cols per chunk
        c1h = {}
        rm1h = {}
        for a in range(NX if upto != "A" else 0):
            for cb in range(2):
                kps = psp.tile([L, CW], F32, tag="kps", bufs=2)  # 2 banks
                kpsh = psp.tile([M, CW], F32, tag="kpsh", bufs=2)
                for blk in range(2):
                    rsl = slice(cb * CW + blk * 512, cb * CW + (blk + 1) * 512)
                    osl = slice(blk * 512, (blk + 1) * 512)
                    nc.tensor.matmul(
                        kps[:, osl],
                        LH[:, a * L : (a + 1) * L],
                        RH[:, rsl],
                    )
                    nc.tensor.matmul(
                        kpsh[:, osl],
                        LH[:, a * L + 1 : (a + 1) * L],
                        RH[:, rsl],
                    )
                ke = workp.tile([L, CW], F32, tag="ke", bufs=3)
                nc.scalar.activation(ke, kps, AF.Exp)
                kesh = workp.tile([M, CW], F32, tag="kesh", bufs=2)
                nc.scalar.activation(kesh, kpsh, AF.Exp)
                ed = workp.tile([M, CW], F32, tag="ed", bufs=2)
                nc.vector.tensor_sub(ed, kesh, ke[0:M, :])
                edv = ed.rearrange("p (b j) -> p b j", b=NY // 2)
                diff = workp.tile([M, CWM], F32, tag="diff", bufs=2)
                nc.vector.tensor_sub(
                    diff.rearrange("p (b j) -> p b j", b=NY // 2),
                    edv[:, :, 1:L], edv[:, :, 0:M],
                )
                # qb = (diff/sqrt(192))^2 ; c1m1 = diff/8 + qb (fp16)
                # rec = 1/(1+c1m1) ; n = qb + c1m1 ; rm1 = -n*rec (fp16)
                qb = workp.tile([M, CWM], F32, tag="qb", bufs=2)
                nc.scalar.activation(
                    qb, diff, AF.Square, scale=1.0 / np.sqrt(192.0)
                )
                c1 = workp.tile([M, CWM], F16, tag="c1m1", bufs=4)
                nc.vector.scalar_tensor_tensor(
                    c1, diff, 0.125, qb, AL.mult, AL.add
                )
                c1f = workp.tile([M, CWM], F32, tag="c1f", bufs=2)
                nc.scalar.activation(c1f, c1, AF.Copy, bias=1.0)
                rec = workp.tile([M, CWM], F32, tag="rec", bufs=2)
                nc.vector.reciprocal(rec, c1f)
                n = workp.tile([M, CWM], F32, tag="n", bufs=2)
                nc.gpsimd.tensor_add(n, qb, c1)
                rm = workp.tile([M, CWM], F16, tag="rm1", bufs=4)
                nc.vector.scalar_tensor_tensor(
                    rm, n, -1.0, rec, AL.mult, AL.mult
                )
                c1h[(a, cb)] = c1
                rm1h[(a, cb)] = rm

        # ---- Phase D: DRAM bounce into skewed pair-major layout ----
        # CHC/CHR [128, SLOTS*32] fp16; partition 32g+16a+b; group g covers
        # coarse cols 32g..32g+31 (g=3: 31 + zero pad); slot k holds coarse
        # row k-g. Stores are contiguous row-major (127 descs of 4KB); loads
        # gather the per-group column slices.
        if upto == "C":
            nc.sync.dma_start(
                out=out_t[:, :], in_=c1h[(1, 1)][0:NP, 0:2].bitcast(F32)
            )
        if upto in ("A", "C"):
            nc.finalize_after_pools = True  # sentinel unused; early build
        else:
            dramp = ctx.enter_context(
                tc.tile_pool(name="dramp", bufs=1, space="DRAM")
            )
            # lo/hi split by slot so phase E can start while hi loads/expands
            SLO = 66  # slots 0..65 in lo tiles, 66..SLOTS-1 in hi
            CHt = {}
            for nm2, part, nsl in (("c", "lo", SLO), ("c", "hi", SLOTS - SLO),
                                   ("r", "lo", SLO), ("r", "hi", SLOTS - SLO)):
                t = chp.tile([4 * NP, nsl * 32], F16, tag=f"CH{nm2}_{part}")
                nc.vector.memset(t, 0.0)
                CHt[(nm2, part)] = t
            di = 0
            drt = {}
            for a in range(NX):
                for nm, arrs in (("c1", c1h), ("rm", rm1h)):
                    d = dramp.tile([M, W], F16, tag=f"d_{nm}{a}")
                    for cb in range(2):
                        rings[di % 3].dma_start(
                            out=d[:, cb * CWM : (cb + 1) * CWM],
                            in_=arrs[(a, cb)][0:M, :],
                        )
                        di += 1
                    drt[(nm, a)] = d
            for part in ("lo", "hi"):
                for g in range(4):
                    w = 32 if g < 3 else 31
                    # slot k holds coarse row k-g; lo covers slots g..SLO-1
                    if part == "lo":
                        r0, r1 = 0, SLO - g        # coarse rows
                        base = 0
                    else:
                        r0, r1 = SLO - g, M
                        base = SLO
                    for a in range(NX):
                        for nm2, nm in (("c", "c1"), ("r", "rm")):
                            CH = CHt[(nm2, part)]
                            d = drt[(nm, a)]
                            dst = CH[
                                32 * g + 16 * a : 32 * g + 16 * a + 16,
                                (g + r0 - base) * 32 : (g + r1 - base) * 32,
                            ].rearrange("p (r c) -> p r c", c=32)[:, :, 0:w]
                            src = d.rearrange("r (q c) -> q r c", q=NY)[
                                :, r0:r1, 32 * g : 32 * g + w
                            ]
                            rings[di % 3].dma_start(out=dst, in_=src)
                            di += 1

        # ---- Phase E: 4-group wavefront ----
        psp_cm.__exit__(None, None, None)
        if upto == "D":
            nc.sync.dma_start(
                out=out_t[:, :], in_=CHt[("c", "hi")][0:NP, 0:2].bitcast(F32)
            )
        if upto in ("A", "C", "D"):
            nstep = 0
        else:
            psE = ctx.enter_context(
                tc.tile_pool(name="psE", bufs=1, space="PSUM")
            )
            # bulk coefficient expansion: fp16 (c-1) -> fp32 c, repeat2
            # lo/hi expansion tiles; hi expands while phase E runs on lo
            pc1_lo = chp.tile([4 * NP, SLO * 64], F32, tag="PC1_lo")
            pc1_hi = chp.tile([4 * NP, (SLOTS - SLO) * 64], F32, tag="PC1_hi")
            pr_lo = chp.tile([4 * NP, SLO * 64], F32, tag="PR_lo")
            pr_hi = chp.tile([4 * NP, (SLOTS - SLO) * 64], F32, tag="PR_hi")
            PC1 = {"lo": pc1_lo, "hi": pc1_hi}
            PR = {"lo": pr_lo, "hi": pr_hi}
            for part, bounds in (("lo", (0, 8, 24, SLO)),
                                 ("hi", (0, SLOTS - SLO))):
                for ci in range(len(bounds) - 1):
                    s0, s1 = bounds[ci], bounds[ci + 1]
                    nc.scalar.activation(
                        PC1[part][:, s0 * 64 : s1 * 64].rearrange(
                            "p (a b) -> p a b", b=2),
                        _rep2(CHt[("c", part)][:, s0 * 32 : s1 * 32]),
                        AF.Copy, bias=1.0,
                    )
                    nc.scalar.activation(
                        PR[part][:, s0 * 64 : s1 * 64].rearrange(
                            "p (a b) -> p a b", b=2),
                        _rep2(CHt[("r", part)][:, s0 * 32 : s1 * 32]),
                        AF.Copy, bias=1.0,
                    )
            GA = constp.tile([4 * NP, 66], F32)
            GB = constp.tile([4 * NP, 66], F32)
            nc.vector.memset(GA, 1.0)
            nc.vector.memset(GB, 1.0)
            B3 = []
            for k in range(3):
                b = psE.tile([4 * NP, 1], F32, tag=f"b3_{k}")
                # initialize boundary buffers to 1.0 via the ones matmul
                nc.tensor.matmul(b, e0row, one1, start=True, stop=False)
                nc.tensor.matmul(
                    b, shp, GA[:, 64:65], start=False, stop=True
                )
                B3.append(b)

        for T in range(1, nstep + 1):
            slot = (T - 1) // 2
            part = "lo" if slot < SLO else "hi"
            sl = slot if part == "lo" else slot - SLO
            exp_c = PC1[part][:, sl * 64 : (sl + 1) * 64]
            exp_r = PR[part][:, sl * 64 : (sl + 1) * 64]
            cur, prv = (GA, GB) if T % 2 == 1 else (GB, GA)
            kb = T % 3
            kread = (T + 1) % 3  # holds boundary produced at step T-2
            # boundary -> cur col 0 (k0 for next step's mul; scan initial)
            nc.vector.tensor_copy(cur[:, 0:1], B3[kread][:, 0:1])
            t64 = ep.tile([4 * NP, 64], F32, tag="t64")
            m = ep.tile([4 * NP, 64], F32, tag="m")
            nc.vector.tensor_mul(t64, exp_r, prv[:, 0:64])
            nc.vector.tensor_sub(m, prv[:, 1:65], t64)
            nc.vector.tensor_tensor_scan(
                cur[:, 1:65], m, exp_c, cur[:, 0:1], AL.add, AL.mult
            )
            # snapshot the scan-end column so PE never reads the G tile
            # (avoids a scan<->PE write-after-read semaphore each step)
            bcol = ep.tile([4 * NP, 1], F32, tag="bcol", bufs=3)
            nc.vector.tensor_copy(bcol, cur[:, 64:65])
            # boundary out via PE shift: B3[kb] = shp^T. @ bcol + e0
            nc.tensor.matmul(B3[kb], e0row, one1, start=True, stop=False)
            nc.tensor.matmul(B3[kb], shp, bcol, start=False, stop=True)

        if nstep > 0:
            final = GB if nstep % 2 == 0 else GA
            nc.sync.dma_start(out=out_t[:, :], in_=final[96:128, 62:63])

    nc.finalize()
    return nc


_CACHE = {}


def _get_nc():
    if "nc" not in _CACHE:
        _CACHE["nc"] = _build()
    return _CACHE["nc"]


def run(xs, ys, trace=False):
    xs = np.ascontiguousarray(np.asarray(xs), dtype=np.float32)
    ys = np.ascontiguousarray(np.asarray(ys), dtype=np.float32)
    assert xs.shape == (16, L, D) and ys.shape == (16, L, D)
    nc = _get_nc()
    idn = np.eye(L, dtype=np.float32)
    shf = np.eye(L, k=-1, dtype=np.float32)
    shp = np.eye(L, k=32, dtype=np.float32)
    in_maps = []
    for c in range(N_CORES):
        in_maps.append(
            {
                "xs": xs[2 * c : 2 * c + 2].reshape(NX * L, D).copy(),
                "ys": ys.reshape(NY * L, D).copy(),
                "idn": idn,
                "shf": shf,
                "shp": shp,
            }
        )
    try:
        res = run_bass_kernel_spmd(nc, in_maps, list(range(N_CORES)), trace=trace)
    except ModuleNotFoundError:
        res = run_bass_kernel_spmd(nc, in_maps, list(range(N_CORES)), trace=False)
    rows = [res.results[c]["out"].reshape(NX, NY) for c in range(N_CORES)]
    out = np.concatenate(rows, axis=0)
    return out, res


def kernel(xs, ys):
    out, _ = run(xs, ys)
    return out


# revision 7
# speedup vs baseline: 1.0416x; 1.0010x over previous
"""Signature-kernel Gram matrix on 8 NeuronCores.

Math per pair (x (128,8), y (128,8)):
  K = exp(x@y.T - 0.5|x|^2 - 0.5|y|^2)           RBF gram, sigma=1
  diff = second mixed finite difference of K      (127,127)
  Goursat PDE on the dyadic-refined fine grid G (255,255), G[0,:]=G[:,0]=1,
    G[i,j] = c1*(G[i-1,j]+G[i,j-1]) - c2*G[i-1,j-1]
    with inc = diff/4 constant on 2x2 fine blocks,
    c1 = 1 + diff/8 + diff^2/192, c2 = 1 - diff^2/192
  answer = G[254,254]

Structure (one core = 32 pairs = 2 local xs x 16 ys):
  A: batched loads, PE transposes, stacked matmul operands
     LH[10,256] = [x^T; -|x|^2/2; 1], RH[10,2048] = [y^T; 1; -|y|^2/2]
     (norms via ACT square + DVE reduce + one PE transpose).
  B: per (x-row a, 8-pair chunk): one K=10 matmul for the exponent AND
     one for the row-shifted exponent (lhsT column-offset by 1), ACT exp;
     diff = col-diff of (Ksh - K) on DVE.
  C: c1-1 and r-1 = c2/c1-1 as fp16 (values are tiny, so fp16 on the
     DELTAS keeps ~1e-5 abs precision); reciprocal on DVE.
  D: flatten to pair-major via DRAM bounce: contiguous row-major stores
     (127 descriptors x 4KB), then strided per-column-block loads into
     pre-skewed CH tiles (lo/hi slot split so phase E starts early).
     Layout: partition 32g+16a+b owns pair (a,b), column block g; slot k
     of the CH array holds coarse row k-g (skew 2 rows per group).
  E: bulk-expand c1/r rows (repeat2, +1.0 bias, fp32) once on ACT, then
     254+6 wavefront steps. Step T runs block-group g at fine row T-2g:
     DVE: copy boundary->cur[:,0], t=r*k0, m=k1-t,
          scan state=(m+state)*c1 over all 128 partitions (scan initial
          = cur[:,0]); PE shifts scan-end columns +32 partitions into a
          rotating PSUM buffer (plus an e0 matmul for group 0's 1.0).
  Output: group 3 partitions, local column 62 = G[254,254] per pair.

Sharding: data-parallel over batch_x: core c owns x rows {2c, 2c+1} x all
16 ys. Host gathers the (16,16) output.
"""

import numpy as np
from contextlib import ExitStack

import concourse.bass as bass
import concourse.bacc as bacc
import concourse.tile as tile
from concourse import mybir
from concourse.bass_utils import run_bass_kernel_spmd

F32 = mybir.dt.float32
F16 = mybir.dt.float16
AL = mybir.AluOpType
AF = mybir.ActivationFunctionType

N_CORES = 8
L = 128          # sequence length
D = 8            # feature dim
NY = 16          # ys per core
NX = 2           # xs per core
NP = NX * NY     # 32 pairs per core
M = L - 1        # 127 coarse grid
G = 2 * M        # 254 fine grid (dyadic order 1)
NSEQ = NX + NY   # 18
NSTEP = G + 6    # wavefront steps (4 groups, skew 2)
SLOTS = NSTEP // 2 + 1  # coarse slots incl skew pad


def _rep2(ap):
    """[P, n] view -> [P, n, 2] with zero-stride inner dim."""
    return bass.AP(tensor=ap.tensor, offset=ap.offset,
                   ap=[ap.ap[0], ap.ap[1], [0, 2]])


def _build(upto="full", nstep=NSTEP):
    nc = bacc.Bacc()
    xs_t = nc.dram_tensor("xs", [NX * L, D], F32, kind="ExternalInput")
    ys_t = nc.dram_tensor("ys", [NY * L, D], F32, kind="ExternalInput")
    idn_t = nc.dram_tensor("idn", [L, L], F32, kind="ExternalInput")
    shf_t = nc.dram_tensor("shf", [L, L], F32, kind="ExternalInput")
    shp_t = nc.dram_tensor("shp", [L, L], F32, kind="ExternalInput")
    out_t = nc.dram_tensor("out", [NP, 1], F32, kind="ExternalOutput")

    rings = None  # set after pools

    with ExitStack() as ctx:
        tc = ctx.enter_context(tile.TileContext(nc))
        constp = ctx.enter_context(tc.tile_pool(name="constp", bufs=1))
        iop = ctx.enter_context(tc.tile_pool(name="iop", bufs=3))
        workp = ctx.enter_context(tc.tile_pool(name="workp", bufs=2))
        chp = ctx.enter_context(tc.tile_pool(name="chp", bufs=1))
        ep = ctx.enter_context(tc.tile_pool(name="ep", bufs=2))

        rings = [nc.sync, nc.scalar, nc.gpsimd]

        # ---- Phase A ----
        idn_s = iop.tile([L, L], F32, tag="idn_s")
        nc.sync.dma_start(out=idn_s, in_=idn_t[:, :])
        idn = constp.tile([L, L], F32)
        nc.vector.tensor_copy(idn, idn_s)
        shp_s = iop.tile([L, L], F32, tag="shp_s")
        nc.gpsimd.dma_start(out=shp_s, in_=shp_t[:, :])
        shp = constp.tile([L, L], F32)
        nc.vector.tensor_copy(shp, shp_s)
        ones8 = constp.tile([D, 1], F32)
        nc.vector.memset(ones8, 1.0)
        one1 = constp.tile([1, 1], F32)
        nc.vector.memset(one1, 1.0)
        e0row = constp.tile([1, L], F32)
        nc.vector.memset(e0row, 0.0)
        nc.vector.memset(e0row[:, 0:32], 1.0)

        # LH rows: 0-7 x^T, 8 = -0.5|x|^2, 9 = ones   (cols: a*L..)
        # RH rows: 0-7 y^T, 8 = ones, 9 = -0.5|y|^2   (cols: b*L..)
        LH = constp.tile([D + 2, NX * L], F32)
        RH = constp.tile([D + 2, NY * L], F32)
        ones2k = constp.tile([1, NY * L], F32)
        nc.vector.memset(ones2k, 1.0)
        # rows 8/9 are written via DMA (compute ops must start at partition 0)
        nc.sync.dma_start(out=LH[D + 1 : D + 2, :], in_=ones2k[:, 0 : NX * L])
        nc.scalar.dma_start(out=RH[D : D + 1, :], in_=ones2k[:, :])

        with tc.tile_pool(name="psA", bufs=1, space="PSUM") as psA, \
             tc.tile_pool(name="awork", bufs=1) as awork:
            # batched sequence loads: [i-partition, (seq, feat)]
            xr_s = awork.tile([L, NX * D], F32, tag="xr_s")
            nc.scalar.dma_start(
                out=xr_s, in_=xs_t.rearrange("(a i) k -> i a k", a=NX)
            )
            xr = awork.tile([L, NX * D], F32, tag="xr")
            nc.vector.tensor_copy(xr, xr_s)
            yr_s = awork.tile([L, NY * D], F32, tag="yr_s")
            nc.sync.dma_start(
                out=yr_s, in_=ys_t.rearrange("(b i) k -> i b k", b=NY)
            )
            yr = awork.tile([L, NY * D], F32, tag="yr")
            nc.vector.tensor_copy(yr, yr_s)
            # transposes: 4 per PSUM bank tile, one ACT copy per bank
            psx = psA.tile([D, NX * L], F32, tag="psx", bufs=1)
            for a in range(NX):
                nc.tensor.transpose(
                    psx[:, a * L : (a + 1) * L], xr[:, a * D : (a + 1) * D], idn
                )
            nc.scalar.activation(LH[0:D, :], psx, AF.Copy)
            for yb in range(4):
                psy = psA.tile([D, 4 * L], F32, tag="psy", bufs=2)
                for j in range(4):
                    b = 4 * yb + j
                    nc.tensor.transpose(
                        psy[:, j * L : (j + 1) * L],
                        yr[:, b * D : (b + 1) * D], idn,
                    )
                nc.scalar.activation(
                    RH[0:D, yb * 512 : (yb + 1) * 512], psy, AF.Copy
                )

            # norms: square+reduce in sequence-index layout, one transpose,
            # then DMA rows into LH/RH
            sq = awork.tile([L, (NX + NY) * D], F32, tag="sq")
            nc.scalar.square(sq[:, 0 : NY * D], yr)
            nc.scalar.square(sq[:, NY * D :], xr)
            nr = awork.tile([L, NX + NY], F32, tag="nr")
            nc.vector.tensor_reduce(
                nr, sq.rearrange("p (s k) -> p s k", k=D),
                mybir.AxisListType.X, AL.add,
            )
            nrp = psA.tile([NX + NY, L], F32, tag="nrp", bufs=1)
            nc.tensor.transpose(nrp, nr, idn)
            nrt = awork.tile([NX + NY, L], F32, tag="nrt")
            nc.scalar.activation(nrt, nrp, AF.Copy, scale=-0.5)
            nc.sync.dma_start(out=RH[D + 1 : D + 2, :], in_=nrt[0:NY, :])
            nc.gpsimd.dma_start(out=LH[D : D + 1, :], in_=nrt[NY : NY + NX, :])

        # coefficient staging tiles (per half a): [127p, 16 pairs * 127]
        W = NY * M  # 2032
        c1h = [None, None]
        rm1h = [None, None]

        if upto == "A":
            nc.sync.dma_start(out=out_t[0:2, :], in_=LH[0:2, 0:1])

        # ---- Phases B + C, two 8-pair chunks per half, pipelined ----
        # Exponent AND row-shifted exponent via two matmul sets (f32r);
        # diff = col-diff of (Ksh - K). Engine balance: Pool does the two
        # subs, ACT exps/squares/copies, DVE the coefficient math.
        F32R = mybir.dt.float32r
        psp_cm = tc.tile_pool(name="psp", bufs=1, space="PSUM")
        psp = psp_cm.__enter__()
        CW = NY * L // 2   # 1024 exp cols per chunk
        CWM = W // 2       # 1016 coef cols per chunk
        c1h = {}
        rm1h = {}
        for a in range(NX if upto != "A" else 0):
            for cb in range(2):
                kps = psp.tile([L, CW], F32, tag="kps", bufs=2)  # 2 banks
                kpsh = psp.tile([M, CW], F32, tag="kpsh", bufs=2)
                for blk in range(2):
                    rsl = slice(cb * CW + blk * 512, cb * CW + (blk + 1) * 512)
                    osl = slice(blk * 512, (blk + 1) * 512)
                    nc.tensor.matmul(
                        kps[:, osl],
                        LH[:, a * L : (a + 1) * L],
                        RH[:, rsl],
                    )
                    nc.tensor.matmul(
                        kpsh[:, osl],
                        LH[:, a * L + 1 : (a + 1) * L],
                        RH[:, rsl],
                    )
                ke = workp.tile([L, CW], F32, tag="ke", bufs=3)
                nc.scalar.activation(ke, kps, AF.Exp)
                kesh = workp.tile([M, CW], F32, tag="kesh", bufs=2)
                nc.scalar.activation(kesh, kpsh, AF.Exp)
                ed = workp.tile([M, CW], F32, tag="ed", bufs=2)
                nc.vector.tensor_sub(ed, kesh, ke[0:M, :])
                edv = ed.rearrange("p (b j) -> p b j", b=NY // 2)
                diff = workp.tile([M, CWM], F32, tag="diff", bufs=2)
                nc.vector.tensor_sub(
                    diff.rearrange("p (b j) -> p b j", b=NY // 2),
                    edv[:, :, 1:L], edv[:, :, 0:M],
                )
                # qb = (diff/sqrt(192))^2 ; c1m1 = diff/8 + qb (fp16)
                # rec = 1/(1+c1m1) ; n = qb + c1m1 ; rm1 = -n*rec (fp16)
                qb = workp.tile([M, CWM], F32, tag="qb", bufs=2)
                nc.scalar.activation(
                    qb, diff, AF.Square, scale=1.0 / np.sqrt(192.0)
                )
                c1 = workp.tile([M, CWM], F16, tag="c1m1", bufs=4)
                nc.vector.scalar_tensor_tensor(
                    c1, diff, 0.125, qb, AL.mult, AL.add
                )
                c1f = workp.tile([M, CWM], F32, tag="c1f", bufs=2)
                nc.scalar.activation(c1f, c1, AF.Copy, bias=1.0)
                rec = workp.tile([M, CWM], F32, tag="rec", bufs=2)
                nc.vector.reciprocal(rec, c1f)
                n = workp.tile([M, CWM], F32, tag="n", bufs=2)
                nc.gpsimd.tensor_add(n, qb, c1)
                rm = workp.tile([M, CWM], F16, tag="rm1", bufs=4)
                nc.vector.scalar_tensor_tensor(
                    rm, n, -1.0, rec, AL.mult, AL.mult
                )
                c1h[(a, cb)] = c1
                rm1h[(a, cb)] = rm

        # ---- Phase D: DRAM bounce into skewed pair-major layout ----
        # CHC/CHR [128, SLOTS*32] fp16; partition 32g+16a+b; group g covers
        # coarse cols 32g..32g+31 (g=3: 31 + zero pad); slot k holds coarse
        # row k-g. Stores are contiguous row-major (127 descs of 4KB); loads
        # gather the per-group column slices.
        if upto == "C":
            nc.sync.dma_start(
                out=out_t[:, :], in_=c1h[(1, 1)][0:NP, 0:2].bitcast(F32)
            )
        if upto in ("A", "C"):
            nc.finalize_after_pools = True  # sentinel unused; early build
        else:
            dramp = ctx.enter_context(
                tc.tile_pool(name="dramp", bufs=1, space="DRAM")
            )
            # lo/hi split by slot so phase E can start while hi loads/expands
            SLO = 66  # slots 0..65 in lo tiles, 66..SLOTS-1 in hi
            CHt = {}
            for nm2, part, nsl in (("c", "lo", SLO), ("c", "hi", SLOTS - SLO),
                                   ("r", "lo", SLO), ("r", "hi", SLOTS - SLO)):
                t = chp.tile([4 * NP, nsl * 32], F16, tag=f"CH{nm2}_{part}")
                nc.vector.memset(t, 0.0)
                CHt[(nm2, part)] = t
            di = 0
            drt = {}
            for a in range(NX):
                for nm, arrs in (("c1", c1h), ("rm", rm1h)):
                    d = dramp.tile([M, W], F16, tag=f"d_{nm}{a}")
                    for cb in range(2):
                        rings[di % 3].dma_start(
                            out=d[:, cb * CWM : (cb + 1) * CWM],
                            in_=arrs[(a, cb)][0:M, :],
                        )
                        di += 1
                    drt[(nm, a)] = d
            for part in ("lo", "hi"):
                for g in range(4):
                    w = 32 if g < 3 else 31
                    # slot k holds coarse row k-g; lo covers slots g..SLO-1
                    if part == "lo":
                        r0, r1 = 0, SLO - g        # coarse rows
                        base = 0
                    else:
                        r0, r1 = SLO - g, M
                        base = SLO
                    for a in range(NX):
                        for nm2, nm in (("c", "c1"), ("r", "rm")):
                            CH = CHt[(nm2, part)]
                            d = drt[(nm, a)]
                            dst = CH[
                                32 * g + 16 * a : 32 * g + 16 * a + 16,
                                (g + r0 - base) * 32 : (g + r1 - base) * 32,
                            ].rearrange("p (r c) -> p r c", c=32)[:, :, 0:w]
                            src = d.rearrange("r (q c) -> q r c", q=NY)[
                                :, r0:r1, 32 * g : 32 * g + w
                            ]
                            rings[di % 3].dma_start(out=dst, in_=src)
                            di += 1

        # ---- Phase E: 4-group wavefront ----
        psp_cm.__exit__(None, None, None)
        if upto == "D":
            nc.sync.dma_start(
                out=out_t[:, :], in_=CHt[("c", "hi")][0:NP, 0:2].bitcast(F32)
            )
        if upto in ("A", "C", "D"):
            nstep = 0
        else:
            psE = ctx.enter_context(
                tc.tile_pool(name="psE", bufs=1, space="PSUM")
            )
            # bulk coefficient expansion: fp16 (c-1) -> fp32 c, repeat2
            # lo/hi expansion tiles; hi expands while phase E runs on lo
            pc1_lo = chp.tile([4 * NP, SLO * 64], F32, tag="PC1_lo")
            pc1_hi = chp.tile([4 * NP, (SLOTS - SLO) * 64], F32, tag="PC1_hi")
            pr_lo = chp.tile([4 * NP, SLO * 64], F32, tag="PR_lo")
            pr_hi = chp.tile([4 * NP, (SLOTS - SLO) * 64], F32, tag="PR_hi")
            PC1 = {"lo": pc1_lo, "hi": pc1_hi}
            PR = {"lo": pr_lo, "hi": pr_hi}
            for part, bounds in (("lo", (0, 6, 16, 36, SLO)),
                                 ("hi", (0, SLOTS - SLO))):
                for ci in range(len(bounds) - 1):
                    s0, s1 = bounds[ci], bounds[ci + 1]
                    nc.scalar.activation(
                        PC1[part][:, s0 * 64 : s1 * 64].rearrange(
                            "p (a b) -> p a b", b=2),
                        _rep2(CHt[("c", part)][:, s0 * 32 : s1 * 32]),
                        AF.Copy, bias=1.0,
                    )
                    nc.scalar.activation(
                        PR[part][:, s0 * 64 : s1 * 64].rearrange(
                            "p (a b) -> p a b", b=2),
                        _rep2(CHt[("r", part)][:, s0 * 32 : s1 * 32]),
                        AF.Copy, bias=1.0,
                    )
            GA = constp.tile([4 * NP, 66], F32)
            GB = constp.tile([4 * NP, 66], F32)
            nc.vector.memset(GA, 1.0)
            nc.vector.memset(GB, 1.0)
            B3 = []
            for k in range(3):
                b = psE.tile([4 * NP, 1], F32, tag=f"b3_{k}")
                # initialize boundary buffers to 1.0 via the ones matmul
                nc.tensor.matmul(b, e0row, one1, start=True, stop=False)
                nc.tensor.matmul(
                    b, shp, GA[:, 64:65], start=False, stop=True
                )
                B3.append(b)

        for T in range(1, nstep + 1):
            slot = (T - 1) // 2
            part = "lo" if slot < SLO else "hi"
            sl = slot if part == "lo" else slot - SLO
            exp_c = PC1[part][:, sl * 64 : (sl + 1) * 64]
            exp_r = PR[part][:, sl * 64 : (sl + 1) * 64]
            cur, prv = (GA, GB) if T % 2 == 1 else (GB, GA)
            kb = T % 3
            kread = (T + 1) % 3  # holds boundary produced at step T-2
            # boundary -> cur col 0 (k0 for next step's mul; scan initial)
            nc.vector.tensor_copy(cur[:, 0:1], B3[kread][:, 0:1])
            t64 = ep.tile([4 * NP, 64], F32, tag="t64")
            m = ep.tile([4 * NP, 64], F32, tag="m")
            nc.vector.tensor_mul(t64, exp_r, prv[:, 0:64])
            nc.vector.tensor_sub(m, prv[:, 1:65], t64)
            nc.vector.tensor_tensor_scan(
                cur[:, 1:65], m, exp_c, cur[:, 0:1], AL.add, AL.mult
            )
            # snapshot the scan-end column so PE never reads the G tile
            # (avoids a scan<->PE write-after-read semaphore each step)
            bcol = ep.tile([4 * NP, 1], F32, tag="bcol", bufs=3)
            nc.vector.tensor_copy(bcol, cur[:, 64:65])
            # boundary out via PE shift: B3[kb] = shp^T. @ bcol + e0
            nc.tensor.matmul(B3[kb], e0row, one1, start=True, stop=False)
            nc.tensor.matmul(B3[kb], shp, bcol, start=False, stop=True)

        if nstep > 0:
            final = GB if nstep % 2 == 0 else GA
            nc.sync.dma_start(out=out_t[:, :], in_=final[96:128, 62:63])

    nc.finalize()
    return nc


_CACHE = {}


def _get_nc():
    if "nc" not in _CACHE:
        _CACHE["nc"] = _build()
    return _CACHE["nc"]


def run(xs, ys, trace=False):
    xs = np.ascontiguousarray(np.asarray(xs), dtype=np.float32)
    ys = np.ascontiguousarray(np.asarray(ys), dtype=np.float32)
    assert xs.shape == (16, L, D) and ys.shape == (16, L, D)
    nc = _get_nc()
    idn = np.eye(L, dtype=np.float32)
    shf = np.eye(L, k=-1, dtype=np.float32)
    shp = np.eye(L, k=32, dtype=np.float32)
    in_maps = []
    for c in range(N_CORES):
        in_maps.append(
            {
                "xs": xs[2 * c : 2 * c + 2].reshape(NX * L, D).copy(),
                "ys": ys.reshape(NY * L, D).copy(),
                "idn": idn,
                "shf": shf,
                "shp": shp,
            }
        )
    try:
        res = run_bass_kernel_spmd(nc, in_maps, list(range(N_CORES)), trace=trace)
    except ModuleNotFoundError:
        res = run_bass_kernel_spmd(nc, in_maps, list(range(N_CORES)), trace=False)
    rows = [res.results[c]["out"].reshape(NX, NY) for c in range(N_CORES)]
    out = np.concatenate(rows, axis=0)
    return out, res


def kernel(xs, ys):
    out, _ = run(xs, ys)
    return out


# revision 8
# speedup vs baseline: 1.0444x; 1.0027x over previous
"""Signature-kernel Gram matrix on 8 NeuronCores.

Math per pair (x (128,8), y (128,8)):
  K = exp(x@y.T - 0.5|x|^2 - 0.5|y|^2)           RBF gram, sigma=1
  diff = second mixed finite difference of K      (127,127)
  Goursat PDE on the dyadic-refined fine grid G (255,255), G[0,:]=G[:,0]=1,
    G[i,j] = c1*(G[i-1,j]+G[i,j-1]) - c2*G[i-1,j-1]
    with inc = diff/4 constant on 2x2 fine blocks,
    c1 = 1 + diff/8 + diff^2/192, c2 = 1 - diff^2/192
  answer = G[254,254]

Structure (one core = 32 pairs = 2 local xs x 16 ys):
  A: batched loads, PE transposes, stacked matmul operands
     LH[10,256] = [x^T; -|x|^2/2; 1], RH[10,2048] = [y^T; 1; -|y|^2/2]
     (norms via ACT square + DVE reduce + one PE transpose).
  B: per (x-row a, 8-pair chunk): one K=10 matmul for the exponent AND
     one for the row-shifted exponent (lhsT column-offset by 1), ACT exp;
     diff = col-diff of (Ksh - K) on DVE.
  C: c1-1 and r-1 = c2/c1-1 as fp16 (values are tiny, so fp16 on the
     DELTAS keeps ~1e-5 abs precision); reciprocal on DVE.
  D: flatten to pair-major via DRAM bounce: contiguous row-major stores
     (127 descriptors x 4KB), then strided per-column-block loads into
     pre-skewed CH tiles (lo/hi slot split so phase E starts early).
     Layout: partition 32g+16a+b owns pair (a,b), column block g; slot k
     of the CH array holds coarse row k-g (skew 2 rows per group).
  E: bulk-expand c1/r rows (repeat2, +1.0 bias, fp32) once on ACT, then
     254+6 wavefront steps. Step T runs block-group g at fine row T-2g:
     DVE: copy boundary->cur[:,0], t=r*k0, m=k1-t,
          scan state=(m+state)*c1 over all 128 partitions (scan initial
          = cur[:,0]); PE shifts scan-end columns +32 partitions into a
          rotating PSUM buffer (plus an e0 matmul for group 0's 1.0).
  Output: group 3 partitions, local column 62 = G[254,254] per pair.

Sharding: data-parallel over batch_x: core c owns x rows {2c, 2c+1} x all
16 ys. Host gathers the (16,16) output.
"""

import numpy as np
from contextlib import ExitStack

import concourse.bass as bass
import concourse.bacc as bacc
import concourse.tile as tile
from concourse import mybir
from concourse.bass_utils import run_bass_kernel_spmd

F32 = mybir.dt.float32
F16 = mybir.dt.float16
AL = mybir.AluOpType
AF = mybir.ActivationFunctionType

N_CORES = 8
L = 128          # sequence length
D = 8            # feature dim
NY = 16          # ys per core
NX = 2           # xs per core
NP = NX * NY     # 32 pairs per core
M = L - 1        # 127 coarse grid
G = 2 * M        # 254 fine grid (dyadic order 1)
NSEQ = NX + NY   # 18
NSTEP = G + 6    # wavefront steps (4 groups, skew 2)
SLOTS = NSTEP // 2 + 1  # coarse slots incl skew pad


def _rep2(ap):
    """[P, n] view -> [P, n, 2] with zero-stride inner dim."""
    return bass.AP(tensor=ap.tensor, offset=ap.offset,
                   ap=[ap.ap[0], ap.ap[1], [0, 2]])


def _build(upto="full", nstep=NSTEP):
    nc = bacc.Bacc()
    xs_t = nc.dram_tensor("xs", [NX * L, D], F32, kind="ExternalInput")
    ys_t = nc.dram_tensor("ys", [NY * L, D], F32, kind="ExternalInput")
    idn_t = nc.dram_tensor("idn", [L, L], F32, kind="ExternalInput")
    shf_t = nc.dram_tensor("shf", [L, L], F32, kind="ExternalInput")
    shp_t = nc.dram_tensor("shp", [L, L], F32, kind="ExternalInput")
    out_t = nc.dram_tensor("out", [NP, 1], F32, kind="ExternalOutput")

    rings = None  # set after pools

    with ExitStack() as ctx:
        tc = ctx.enter_context(tile.TileContext(nc))
        constp = ctx.enter_context(tc.tile_pool(name="constp", bufs=1))
        iop = ctx.enter_context(tc.tile_pool(name="iop", bufs=3))
        workp = ctx.enter_context(tc.tile_pool(name="workp", bufs=2))
        chp = ctx.enter_context(tc.tile_pool(name="chp", bufs=1))
        ep = ctx.enter_context(tc.tile_pool(name="ep", bufs=2))

        rings = [nc.sync, nc.scalar, nc.gpsimd]

        # ---- Phase A ----
        idn_s = iop.tile([L, L], F32, tag="idn_s")
        nc.sync.dma_start(out=idn_s, in_=idn_t[:, :])
        idn = constp.tile([L, L], F32)
        nc.vector.tensor_copy(idn, idn_s)
        shp_s = iop.tile([L, L], F32, tag="shp_s")
        nc.gpsimd.dma_start(out=shp_s, in_=shp_t[:, :])
        shp = constp.tile([L, L], F32)
        nc.vector.tensor_copy(shp, shp_s)
        ones8 = constp.tile([D, 1], F32)
        nc.vector.memset(ones8, 1.0)
        one1 = constp.tile([1, 1], F32)
        nc.vector.memset(one1, 1.0)
        e0row = constp.tile([1, L], F32)
        nc.vector.memset(e0row, 0.0)
        nc.vector.memset(e0row[:, 0:32], 1.0)

        # LH rows: 0-7 x^T, 8 = -0.5|x|^2, 9 = ones   (cols: a*L..)
        # RH rows: 0-7 y^T, 8 = ones, 9 = -0.5|y|^2   (cols: b*L..)
        LH = constp.tile([D + 2, NX * L], F32)
        RH = constp.tile([D + 2, NY * L], F32)
        ones2k = constp.tile([1, NY * L], F32)
        nc.vector.memset(ones2k, 1.0)
        # rows 8/9 are written via DMA (compute ops must start at partition 0)
        nc.sync.dma_start(out=LH[D + 1 : D + 2, :], in_=ones2k[:, 0 : NX * L])
        nc.scalar.dma_start(out=RH[D : D + 1, :], in_=ones2k[:, :])

        with tc.tile_pool(name="psA", bufs=1, space="PSUM") as psA, \
             tc.tile_pool(name="awork", bufs=1) as awork:
            # batched sequence loads: [i-partition, (seq, feat)]
            xr_s = awork.tile([L, NX * D], F32, tag="xr_s")
            nc.scalar.dma_start(
                out=xr_s, in_=xs_t.rearrange("(a i) k -> i a k", a=NX)
            )
            xr = awork.tile([L, NX * D], F32, tag="xr")
            nc.vector.tensor_copy(xr, xr_s)
            yr_s = awork.tile([L, NY * D], F32, tag="yr_s")
            nc.sync.dma_start(
                out=yr_s, in_=ys_t.rearrange("(b i) k -> i b k", b=NY)
            )
            yr = awork.tile([L, NY * D], F32, tag="yr")
            nc.vector.tensor_copy(yr, yr_s)
            # transposes: 4 per PSUM bank tile, one ACT copy per bank
            psx = psA.tile([D, NX * L], F32, tag="psx", bufs=1)
            for a in range(NX):
                nc.tensor.transpose(
                    psx[:, a * L : (a + 1) * L], xr[:, a * D : (a + 1) * D], idn
                )
            nc.scalar.activation(LH[0:D, :], psx, AF.Copy)
            for yb in range(4):
                psy = psA.tile([D, 4 * L], F32, tag="psy", bufs=2)
                for j in range(4):
                    b = 4 * yb + j
                    nc.tensor.transpose(
                        psy[:, j * L : (j + 1) * L],
                        yr[:, b * D : (b + 1) * D], idn,
                    )
                nc.scalar.activation(
                    RH[0:D, yb * 512 : (yb + 1) * 512], psy, AF.Copy
                )

            # norms: square+reduce in sequence-index layout, one transpose,
            # then DMA rows into LH/RH
            sq = awork.tile([L, (NX + NY) * D], F32, tag="sq")
            nc.scalar.square(sq[:, 0 : NY * D], yr)
            nc.scalar.square(sq[:, NY * D :], xr)
            nr = awork.tile([L, NX + NY], F32, tag="nr")
            nc.vector.tensor_reduce(
                nr, sq.rearrange("p (s k) -> p s k", k=D),
                mybir.AxisListType.X, AL.add,
            )
            nrp = psA.tile([NX + NY, L], F32, tag="nrp", bufs=1)
            nc.tensor.transpose(nrp, nr, idn)
            nrt = awork.tile([NX + NY, L], F32, tag="nrt")
            nc.scalar.activation(nrt, nrp, AF.Copy, scale=-0.5)
            nc.sync.dma_start(out=RH[D + 1 : D + 2, :], in_=nrt[0:NY, :])
            nc.gpsimd.dma_start(out=LH[D : D + 1, :], in_=nrt[NY : NY + NX, :])

        # coefficient staging tiles (per half a): [127p, 16 pairs * 127]
        W = NY * M  # 2032
        c1h = [None, None]
        rm1h = [None, None]

        if upto == "A":
            nc.sync.dma_start(out=out_t[0:2, :], in_=LH[0:2, 0:1])

        # ---- Phases B + C, two 8-pair chunks per half, pipelined ----
        # Exponent AND row-shifted exponent via two matmul sets (f32r);
        # diff = col-diff of (Ksh - K). Engine balance: Pool does the two
        # subs, ACT exps/squares/copies, DVE the coefficient math.
        F32R = mybir.dt.float32r
        psp_cm = tc.tile_pool(name="psp", bufs=1, space="PSUM")
        psp = psp_cm.__enter__()
        CW = NY * L // 2   # 1024 exp cols per chunk
        CWM = W // 2       # 1016 coef cols per chunk
        c1h = {}
        rm1h = {}
        for a in range(NX if upto != "A" else 0):
            for cb in range(2):
                kps = psp.tile([L, CW], F32, tag="kps", bufs=2)  # 2 banks
                kpsh = psp.tile([M, CW], F32, tag="kpsh", bufs=2)
                for blk in range(2):
                    rsl = slice(cb * CW + blk * 512, cb * CW + (blk + 1) * 512)
                    osl = slice(blk * 512, (blk + 1) * 512)
                    nc.tensor.matmul(
                        kps[:, osl],
                        LH[:, a * L : (a + 1) * L],
                        RH[:, rsl],
                    )
                    nc.tensor.matmul(
                        kpsh[:, osl],
                        LH[:, a * L + 1 : (a + 1) * L],
                        RH[:, rsl],
                    )
                ke = workp.tile([L, CW], F32, tag="ke", bufs=3)
                nc.scalar.activation(ke, kps, AF.Exp)
                kesh = workp.tile([M, CW], F32, tag="kesh", bufs=2)
                nc.scalar.activation(kesh, kpsh, AF.Exp)
                ed = workp.tile([M, CW], F32, tag="ed", bufs=2)
                nc.vector.tensor_sub(ed, kesh, ke[0:M, :])
                edv = ed.rearrange("p (b j) -> p b j", b=NY // 2)
                diff = workp.tile([M, CWM], F32, tag="diff", bufs=2)
                nc.vector.tensor_sub(
                    diff.rearrange("p (b j) -> p b j", b=NY // 2),
                    edv[:, :, 1:L], edv[:, :, 0:M],
                )
                # qb = (diff/sqrt(192))^2 ; c1m1 = diff/8 + qb (fp16)
                # rec = 1/(1+c1m1) ; n = qb + c1m1 ; rm1 = -n*rec (fp16)
                qb = workp.tile([M, CWM], F32, tag="qb", bufs=2)
                nc.scalar.activation(
                    qb, diff, AF.Square, scale=1.0 / np.sqrt(192.0)
                )
                c1 = workp.tile([M, CWM], F16, tag="c1m1", bufs=4)
                nc.vector.scalar_tensor_tensor(
                    c1, diff, 0.125, qb, AL.mult, AL.add
                )
                c1f = workp.tile([M, CWM], F32, tag="c1f", bufs=2)
                nc.scalar.activation(c1f, c1, AF.Copy, bias=1.0)
                rec = workp.tile([M, CWM], F32, tag="rec", bufs=2)
                nc.vector.reciprocal(rec, c1f)
                n = workp.tile([M, CWM], F32, tag="n", bufs=2)
                nc.gpsimd.tensor_add(n, qb, c1)
                rm = workp.tile([M, CWM], F16, tag="rm1", bufs=4)
                nc.vector.scalar_tensor_tensor(
                    rm, n, -1.0, rec, AL.mult, AL.mult
                )
                c1h[(a, cb)] = c1
                rm1h[(a, cb)] = rm

        # ---- Phase D: DRAM bounce into skewed pair-major layout ----
        # CHC/CHR [128, SLOTS*32] fp16; partition 32g+16a+b; group g covers
        # coarse cols 32g..32g+31 (g=3: 31 + zero pad); slot k holds coarse
        # row k-g. Stores are contiguous row-major (127 descs of 4KB); loads
        # gather the per-group column slices.
        if upto == "C":
            nc.sync.dma_start(
                out=out_t[:, :], in_=c1h[(1, 1)][0:NP, 0:2].bitcast(F32)
            )
        if upto in ("A", "C"):
            nc.finalize_after_pools = True  # sentinel unused; early build
        else:
            dramp = ctx.enter_context(
                tc.tile_pool(name="dramp", bufs=1, space="DRAM")
            )
            # lo/hi split by slot so phase E can start while hi loads/expands
            SLO = 66  # slots 0..65 in lo tiles, 66..SLOTS-1 in hi
            CHt = {}
            for nm2, part, nsl in (("c", "lo", SLO), ("c", "hi", SLOTS - SLO),
                                   ("r", "lo", SLO), ("r", "hi", SLOTS - SLO)):
                t = chp.tile([4 * NP, nsl * 32], F16, tag=f"CH{nm2}_{part}")
                nc.vector.memset(t, 0.0)
                CHt[(nm2, part)] = t
            di = 0
            drt = {}
            for a in range(NX):
                for nm, arrs in (("c1", c1h), ("rm", rm1h)):
                    d = dramp.tile([M, W], F16, tag=f"d_{nm}{a}")
                    for cb in range(2):
                        rings[di % 3].dma_start(
                            out=d[:, cb * CWM : (cb + 1) * CWM],
                            in_=arrs[(a, cb)][0:M, :],
                        )
                        di += 1
                    drt[(nm, a)] = d
            for part in ("lo", "hi"):
                for g in range(4):
                    w = 32 if g < 3 else 31
                    # slot k holds coarse row k-g; lo covers slots g..SLO-1
                    if part == "lo":
                        r0, r1 = 0, SLO - g        # coarse rows
                        base = 0
                    else:
                        r0, r1 = SLO - g, M
                        base = SLO
                    for a in range(NX):
                        for nm2, nm in (("c", "c1"), ("r", "rm")):
                            CH = CHt[(nm2, part)]
                            d = drt[(nm, a)]
                            dst = CH[
                                32 * g + 16 * a : 32 * g + 16 * a + 16,
                                (g + r0 - base) * 32 : (g + r1 - base) * 32,
                            ].rearrange("p (r c) -> p r c", c=32)[:, :, 0:w]
                            src = d.rearrange("r (q c) -> q r c", q=NY)[
                                :, r0:r1, 32 * g : 32 * g + w
                            ]
                            rings[di % 3].dma_start(out=dst, in_=src)
                            di += 1

        # ---- Phase E: 4-group wavefront ----
        psp_cm.__exit__(None, None, None)
        if upto == "D":
            nc.sync.dma_start(
                out=out_t[:, :], in_=CHt[("c", "hi")][0:NP, 0:2].bitcast(F32)
            )
        if upto in ("A", "C", "D"):
            nstep = 0
        else:
            psE = ctx.enter_context(
                tc.tile_pool(name="psE", bufs=1, space="PSUM")
            )
            # bulk coefficient expansion: fp16 (c-1) -> fp32 c, repeat2
            # lo/hi expansion tiles; hi expands while phase E runs on lo
            pc1_lo = chp.tile([4 * NP, SLO * 64], F32, tag="PC1_lo")
            pc1_hi = chp.tile([4 * NP, (SLOTS - SLO) * 64], F32, tag="PC1_hi")
            pr_lo = chp.tile([4 * NP, SLO * 64], F32, tag="PR_lo")
            pr_hi = chp.tile([4 * NP, (SLOTS - SLO) * 64], F32, tag="PR_hi")
            PC1 = {"lo": pc1_lo, "hi": pc1_hi}
            PR = {"lo": pr_lo, "hi": pr_hi}
            for part, bounds in (("lo", (0, 1, 3, 7, 15, 27, 43, SLO)),
                                 ("hi", (0, SLOTS - SLO))):
                for ci in range(len(bounds) - 1):
                    s0, s1 = bounds[ci], bounds[ci + 1]
                    nc.scalar.activation(
                        PC1[part][:, s0 * 64 : s1 * 64].rearrange(
                            "p (a b) -> p a b", b=2),
                        _rep2(CHt[("c", part)][:, s0 * 32 : s1 * 32]),
                        AF.Copy, bias=1.0,
                    )
                    nc.scalar.activation(
                        PR[part][:, s0 * 64 : s1 * 64].rearrange(
                            "p (a b) -> p a b", b=2),
                        _rep2(CHt[("r", part)][:, s0 * 32 : s1 * 32]),
                        AF.Copy, bias=1.0,
                    )
            GA = constp.tile([4 * NP, 66], F32)
            GB = constp.tile([4 * NP, 66], F32)
            nc.vector.memset(GA, 1.0)
            nc.vector.memset(GB, 1.0)
            B3 = []
            for k in range(3):
                b = psE.tile([4 * NP, 1], F32, tag=f"b3_{k}")
                # initialize boundary buffers to 1.0 via the ones matmul
                nc.tensor.matmul(b, e0row, one1, start=True, stop=False)
                nc.tensor.matmul(
                    b, shp, GA[:, 64:65], start=False, stop=True
                )
                B3.append(b)

        for T in range(1, nstep + 1):
            slot = (T - 1) // 2
            part = "lo" if slot < SLO else "hi"
            sl = slot if part == "lo" else slot - SLO
            exp_c = PC1[part][:, sl * 64 : (sl + 1) * 64]
            exp_r = PR[part][:, sl * 64 : (sl + 1) * 64]
            cur, prv = (GA, GB) if T % 2 == 1 else (GB, GA)
            kb = T % 3
            kread = (T + 1) % 3  # holds boundary produced at step T-2
            # boundary -> cur col 0 (k0 for next step's mul; scan initial)
            nc.vector.tensor_copy(cur[:, 0:1], B3[kread][:, 0:1])
            t64 = ep.tile([4 * NP, 64], F32, tag="t64")
            m = ep.tile([4 * NP, 64], F32, tag="m")
            nc.vector.tensor_mul(t64, exp_r, prv[:, 0:64])
            nc.vector.tensor_sub(m, prv[:, 1:65], t64)
            nc.vector.tensor_tensor_scan(
                cur[:, 1:65], m, exp_c, cur[:, 0:1], AL.add, AL.mult
            )
            # snapshot the scan-end column so PE never reads the G tile
            # (avoids a scan<->PE write-after-read semaphore each step)
            bcol = ep.tile([4 * NP, 1], F32, tag="bcol", bufs=3)
            nc.vector.tensor_copy(bcol, cur[:, 64:65])
            # boundary out via PE shift: B3[kb] = shp^T. @ bcol + e0
            nc.tensor.matmul(B3[kb], e0row, one1, start=True, stop=False)
            nc.tensor.matmul(B3[kb], shp, bcol, start=False, stop=True)

        if nstep > 0:
            final = GB if nstep % 2 == 0 else GA
            nc.sync.dma_start(out=out_t[:, :], in_=final[96:128, 62:63])

    nc.finalize()
    return nc


_CACHE = {}


def _get_nc():
    if "nc" not in _CACHE:
        _CACHE["nc"] = _build()
    return _CACHE["nc"]


def run(xs, ys, trace=False):
    xs = np.ascontiguousarray(np.asarray(xs), dtype=np.float32)
    ys = np.ascontiguousarray(np.asarray(ys), dtype=np.float32)
    assert xs.shape == (16, L, D) and ys.shape == (16, L, D)
    nc = _get_nc()
    idn = np.eye(L, dtype=np.float32)
    shf = np.eye(L, k=-1, dtype=np.float32)
    shp = np.eye(L, k=32, dtype=np.float32)
    in_maps = []
    for c in range(N_CORES):
        in_maps.append(
            {
                "xs": xs[2 * c : 2 * c + 2].reshape(NX * L, D).copy(),
                "ys": ys.reshape(NY * L, D).copy(),
                "idn": idn,
                "shf": shf,
                "shp": shp,
            }
        )
    try:
        res = run_bass_kernel_spmd(nc, in_maps, list(range(N_CORES)), trace=trace)
    except ModuleNotFoundError:
        res = run_bass_kernel_spmd(nc, in_maps, list(range(N_CORES)), trace=False)
    rows = [res.results[c]["out"].reshape(NX, NY) for c in range(N_CORES)]
    out = np.concatenate(rows, axis=0)
    return out, res


def kernel(xs, ys):
    out, _ = run(xs, ys)
    return out
